# revision 11
# baseline (speedup 1.0000x reference)
"""Modulated deformable conv (DCNv2-style) Trainium2 Bass kernel.

Batch data-parallel over 8 NeuronCores (1 batch element per core).

Per-core pipeline:
  1. fuse 1x1 conv (PE)  -> x, kept as CHW padded in SBUF (X2, with a
     col-shifted duplicate on partitions 64-127 for K-stacked conv taps)
     and as NHWC 2-pixel tokens in HBM (xtok) for gathering.
  2. dy/dx/mod 3x3 convs (PE, 3 pair-slots K=128 + 3 single-slots K=64).
  3. map pipeline (DVE/ACT): floor flags, fracs, modulated corner-weight
     maps CY0/CY1, int16 gather token indices.
  4. dma_gather (transpose=True, 256B tokens = 2px x 64ch bf16): for each
     kernel tap k and corner row y, V[128=(2px,64ch), npix].
  5. corner weights replicated across 128 partitions via PE selector
     matmuls; prod = V * Wrep (DVE); final contraction on PE with w_reg
     folded lhsT (sign/2x-sigmoid folded in host-side).

Column ordering note: gather list position n maps to map-column
sigma(n) = (n%16)*S + n//16  (S = chunk_cols/16) because indices are
stored 16-partition-wrapped with a contiguous inner dim. sigma is applied
at 3 AP sites: the index wrap DMA, the repl-matmul rhs view, and the
final PSUM->SBUF unpermute copy.
"""
import sys

sys.path.insert(0, "/opt/trn_rl_repo")

from contextlib import ExitStack

import numpy as np

import concourse.bass as bass
import concourse.bacc as bacc
import concourse.mybir as mybir
from concourse.tile import TileContext
from concourse.mybir import AluOpType as Op
from concourse.mybir import ActivationFunctionType as Act

F32 = mybir.dt.float32
BF16 = mybir.dt.bfloat16
I16 = mybir.dt.int16

W = 128
C = 64
K2 = 9
PADW = 132


def _shape_consts(H):
    NPX = H * W
    NCH = 8
    CH = NPX // NCH          # pixels per chunk
    RPC = CH // W            # image rows per chunk
    S = CH // 16             # wrap cols per chunk slot
    PADH = H + 4
    NTOK = PADH * PADW
    NSPL = max(1, CH // 512)
    SPL = CH // NSPL         # matmul N per split (<=512)
    return NPX, NCH, CH, RPC, S, PADH, NTOK, NSPL, SPL


def build_nc(H=128, num_devices=8):
    NPX, NCH, CH, RPC, S, PADH, NTOK, NSPL, SPL = _shape_consts(H)
    RSPL = SPL // W                      # image rows per split
    NTOKP = ((NTOK + 2047) // 2048) * 2048

    es = ExitStack()
    nc = bacc.Bacc("TRN2", target_bir_lowering=False, debug=False,
                   num_devices=num_devices)

    x_img = nc.dram_tensor("x_img", [C, NPX], F32, kind="ExternalInput")
    x_cont = nc.dram_tensor("x_cont", [C, NPX], F32, kind="ExternalInput")
    out = nc.dram_tensor("out", [C, NPX], F32, kind="ExternalOutput")

    fuse_lhsT = nc.dram_tensor("fuse_lhsT", [128, 64], BF16, kind="ExternalInput")
    pw, sw = {}, {}
    for q in ("dy", "dx", "mod"):
        for ty in range(3):
            pw[(q, ty)] = nc.dram_tensor(f"pw_{q}_{ty}", [128, 32], BF16,
                                         kind="ExternalInput")
            sw[(q, ty)] = nc.dram_tensor(f"sw_{q}_{ty}", [64, 32], BF16,
                                         kind="ExternalInput")
    dcols = {}
    for nm in ("bias_dy", "bias_dx", "bias_mod", "xw_s1", "xw_s2"):
        dcols[nm] = nc.dram_tensor(nm, [128, 1], F32, kind="ExternalInput")
    basetok = nc.dram_tensor("basetok", [128, 2 * CH], F32, kind="ExternalInput")
    sel = nc.dram_tensor("sel", [128, K2 * 128], BF16, kind="ExternalInput")
    reg0 = nc.dram_tensor("reg0", [128, K2 * 64], BF16, kind="ExternalInput")
    reg1 = nc.dram_tensor("reg1", [128, K2 * 64], BF16, kind="ExternalInput")

    xtok = nc.dram_tensor("xtok", [NTOKP, 128], BF16, kind="Internal")
    xtok3 = xtok.ap()[0:NTOK, :].rearrange("(a b) e -> a b e", b=PADW)

    MM = lambda *a, **k: nc.tensor.matmul(*a, **k)

    with TileContext(nc) as tc:
        pconst = es.enter_context(tc.tile_pool(name="pconst", bufs=1))
        pp = es.enter_context(tc.tile_pool(name="pp", bufs=1))

        # ---- stage constants
        fuse_w = pconst.tile([128, 64], BF16)
        nc.sync.dma_start(fuse_w[:], fuse_lhsT.ap())
        conv_w = {}
        for q in ("dy", "dx", "mod"):
            for ty in range(3):
                tP = pconst.tile([128, 32], BF16, name=f"cwp_{q}{ty}")
                nc.sync.dma_start(tP[:], pw[(q, ty)].ap())
                tS = pconst.tile([64, 32], BF16, name=f"cws_{q}{ty}")
                nc.sync.dma_start(tS[:], sw[(q, ty)].ap())
                conv_w[(q, ty)] = (tP, tS)
        col = {}
        for nm in ("bias_dy", "bias_dx", "bias_mod", "xw_s1", "xw_s2"):
            t = pconst.tile([128, 1], F32, name=f"c_{nm}")
            nc.sync.dma_start(t[:], dcols[nm].ap())
            col[nm] = t
        btok = pconst.tile([128, 2 * CH], F32)
        nc.sync.dma_start(btok[:], basetok.ap())
        sel_sb = pconst.tile([128, K2 * 128], BF16)
        nc.sync.dma_start(sel_sb[:], sel.ap())
        regsb = {}
        for y, t in ((0, reg0), (1, reg1)):
            r = pconst.tile([128, K2 * 64], BF16, name=f"regsb{y}")
            nc.sync.dma_start(r[:], t.ap())
            regsb[y] = r

        CY, IDXT = {}, {}
        WIDX = {y: pp.tile([128, K2 * 8 * S], I16, name=f"widx{y}")
                for y in (0, 1)}

        with tc.tile_pool(name="pX", bufs=1) as pX:
            X2 = pX.tile([128, PADH, PADW], BF16)

            # =============== phase 0 ===============
            with tc.tile_pool(name="pin", bufs=1) as pin, \
                 tc.tile_pool(name="p0ps", bufs=2, space="PSUM") as p0ps:
                instk = pin.tile([128, NPX], BF16)
                nc.gpsimd.dma_start(instk[0:64, :], x_img.ap())
                nc.gpsimd.dma_start(instk[64:128, :], x_cont.ap())

                nc.vector.memset(X2[:, :, :], 0.0)
                zt = pin.tile([128, 2048], BF16)
                nc.vector.memset(zt[:, :], 0.0)
                for r0 in range(0, NTOKP, 2048):
                    nc.sync.dma_start(xtok.ap()[r0:r0 + 2048, :], zt[:, :])

                # fuse conv -> X2 rows 0-63 interior
                for c8 in range(NCH):
                    for j in range(NSPL):
                        ps = p0ps.tile([64, SPL], F32, tag="fuseps")
                        off = c8 * CH + j * SPL
                        MM(ps[:], fuse_w[:, :], instk[:, off:off + SPL],
                           start=True, stop=True)
                        i0 = off // W
                        nc.scalar.copy(X2[0:64, 2 + i0:2 + i0 + RSPL, 2:130],
                                       ps[:].rearrange("p (a b) -> p a b", b=W))

                # transposed fuse -> xtok tokens
                stg = pin.tile([128, RPC * 64], BF16, tag="stg", bufs=2)
                for c8 in range(NCH):
                    for r in range(RPC):
                        i = c8 * RPC + r
                        pst = p0ps.tile([128, 64], F32, tag="fuseT")
                        MM(pst[:], instk[:, i * W:(i + 1) * W], fuse_w[:, :],
                           start=True, stop=True)
                        nc.vector.tensor_copy(stg[:, r * 64:(r + 1) * 64], pst[:])
                    rr = c8 * RPC + 2
                    # first halves: token (y, x=2+j)[0:64] = pixel (y, 2+j)
                    nc.sync.dma_start(
                        xtok3[rr:rr + RPC, 2:130, 0:64].transpose([1, 0, 2]),
                        stg[:, :].rearrange("p (r e) -> p r e", e=64))
                    # second halves: token (y, x=1+j)[64:128] = pixel (y, 2+j)
                    nc.sync.dma_start(
                        xtok3[rr:rr + RPC, 1:129, 64:128].transpose([1, 0, 2]),
                        stg[:, :].rearrange("p (r e) -> p r e", e=64))

                # duplicate col-shifted copy on partitions 64-127
                nc.sync.dma_start(
                    X2[64:128, :, :].rearrange("p a b -> p (a b)")[:, 0:NTOK - 2],
                    X2[0:64, :, :].rearrange("p a b -> p (a b)")[:, 2:NTOK])

            # =============== phase A: convs + maps ===============
            with tc.tile_pool(name="paps", bufs=2, space="PSUM") as paps, \
                 tc.tile_pool(name="pam", bufs=1) as pam:
                for g in range(2):
                    qsb = {}
                    for q in ("dy", "dx", "mod"):
                        qps = paps.tile([128, CH], F32, tag="convps")
                        for cb in range(4):
                            c8 = g * 4 + cb
                            for j in range(NSPL):
                                ist = c8 * RPC + j * RSPL
                                dst = qps[32 * cb:32 * cb + 32,
                                          j * SPL:(j + 1) * SPL]
                                for ty in range(3):
                                    tP, tS = conv_w[(q, ty)]
                                    MM(dst,
                                       tP[:, :],
                                       X2[0:128, 1 + ist + ty:1 + ist + ty + RSPL,
                                          1:1 + W],
                                       start=(ty == 0), stop=False,
                                       tile_position=(0, 32 * cb))
                                    MM(dst,
                                       tS[:, :],
                                       X2[0:64, 1 + ist + ty:1 + ist + ty + RSPL,
                                          2:2 + W],
                                       start=False, stop=(ty == 2),
                                       tile_position=(0, 32 * cb))
                        qs = pam.tile([128, CH], BF16, tag=f"q_{q}",
                                      name=f"qsb_{q}{g}")
                        if q == "mod":
                            nc.scalar.activation(qs[:], qps[:], Act.Sigmoid,
                                                 bias=col["bias_mod"][:], scale=1.0)
                        else:
                            nc.scalar.activation(
                                qs[:], qps[:], Act.Identity,
                                bias=col["bias_dy" if q == "dy" else "bias_dx"][:],
                                scale=1.0)
                        qsb[q] = qs

                    FY = pam.tile([128, CH], BF16, tag="m1")
                    nc.vector.tensor_scalar(FY[:], qsb["dy"][:], 0.0, None, Op.is_lt)
                    FX = pam.tile([128, CH], BF16, tag="m2")
                    nc.vector.tensor_scalar(FX[:], qsb["dx"][:], 0.0, None, Op.is_lt)
                    RY = pam.tile([128, CH], BF16, tag="m3")
                    nc.vector.tensor_tensor(RY[:], qsb["dy"][:], FY[:], Op.add)
                    RX = pam.tile([128, CH], BF16, tag="m4")
                    nc.vector.tensor_tensor(RX[:], qsb["dx"][:], FX[:], Op.add)
                    XW = pam.tile([128, CH], BF16, tag="m5")
                    nc.vector.tensor_scalar(XW[:], RX[:], col["xw_s1"][:],
                                            col["xw_s2"][:], Op.mult, Op.add)
                    WY0N = pam.tile([128, CH], BF16, tag="m6")
                    nc.vector.scalar_tensor_tensor(WY0N[:], RY[:], 1.0,
                                                   qsb["mod"][:],
                                                   Op.subtract, Op.mult)
                    RYM = pam.tile([128, CH], BF16, tag="m7")
                    nc.vector.tensor_tensor(RYM[:], RY[:], qsb["mod"][:], Op.mult)
                    cy0 = pp.tile([128, CH], BF16, name=f"cy0_{g}")
                    nc.vector.tensor_tensor(cy0[:], WY0N[:], XW[:], Op.mult)
                    cy1 = pp.tile([128, CH], BF16, name=f"cy1_{g}")
                    nc.vector.tensor_tensor(cy1[:], RYM[:], XW[:], Op.mult)
                    CY[(g, 0)], CY[(g, 1)] = cy0, cy1

                    T1 = pam.tile([128, CH], F32, tag="m8")
                    nc.vector.scalar_tensor_tensor(T1[:], FY[:], -132.0, FX[:],
                                                   Op.mult, Op.subtract)
                    TOK0 = pam.tile([128, CH], F32, tag="m9")
                    nc.vector.tensor_tensor(TOK0[:], btok[:, g * CH:(g + 1) * CH],
                                            T1[:], Op.add)
                    idx0 = pp.tile([128, CH], I16, name=f"idx0_{g}")
                    nc.vector.tensor_copy(idx0[:], TOK0[:])
                    idx1 = pp.tile([128, CH], I16, name=f"idx1_{g}")
                    nc.vector.tensor_scalar(idx1[:], TOK0[:], 132.0, None, Op.add)
                    IDXT[(g, 0)], IDXT[(g, 1)] = idx0, idx1

                # wrapped indices: WIDX[y][p, slot*S + s] = IDX[row, p*S + s]
                for y in (0, 1):
                    for k in range(K2):
                        for g in range(2):
                            for cb in range(4):
                                slot = (k * 8 + g * 4 + cb) * S
                                src = IDXT[(g, y)][32 * cb + k:32 * cb + k + 1, :]
                                nc.sync.dma_start(
                                    WIDX[y][0:16, slot:slot + S],
                                    src.rearrange("p (a b) -> p a b", b=S))
                    for r8 in range(1, 8):
                        nc.sync.dma_start(WIDX[y][16 * r8:16 * r8 + 16, :],
                                          WIDX[y][0:16, :])

        # =============== phase C: gather / weight / contract ===============
        with tc.tile_pool(name="pcps", bufs=2, space="PSUM") as pcps, \
             tc.tile_pool(name="pops", bufs=1, space="PSUM") as pops, \
             tc.tile_pool(name="pc", bufs=3) as pc:
            for g in range(2):
                for hh in range(2):          # half-group: chunks (2hh, 2hh+1)
                    outp = pops.tile([128, CH], F32, tag="outp", bufs=1)
                    for k in range(K2):
                        for y in (0, 1):
                            v = pc.tile([128, 1, 2 * CH], BF16, tag="vt", bufs=2)
                            islot = (k * 8 + g * 4 + 2 * hh) * S
                            nc.gpsimd.dma_gather(
                                v[:, :, :], xtok.ap(),
                                WIDX[y][:, islot:islot + 2 * S],
                                num_idxs=2 * CH, num_idxs_reg=2 * CH,
                                elem_size=128, transpose=True,
                                single_packet=False)
                            NH = max(1, NSPL // 2)   # 1024-col groups
                            GW = CH // NH            # cols per group
                            NSUB = GW // SPL         # matmul splits per group
                            for ci in range(2):
                                cb = 2 * hh + ci
                                cy = CY[(g, y)]
                                cyv = cy[32 * cb:32 * cb + 18, :].rearrange(
                                    "p (a b) -> p b a", b=S)   # [18, S, 16]
                                for h in range(NH):
                                    wrepp = pcps.tile([128, GW], F32, tag="wrepp")
                                    for u in range(NSUB):
                                        q0 = (h * GW + u * SPL) // 16
                                        MM(wrepp[:, u * SPL:(u + 1) * SPL],
                                           sel_sb[32 * cb:32 * cb + 18,
                                                  k * 128:(k + 1) * 128],
                                           cyv[:, q0:q0 + SPL // 16, :],
                                           start=True, stop=True,
                                           tile_position=(32 * cb, 0),
                                           skip_group_check=True)
                                    wreps = pc.tile([128, GW], BF16, tag="wreps")
                                    nc.scalar.copy(wreps[:], wrepp[:])
                                    prd = pc.tile([128, GW], BF16, tag="prd")
                                    nc.vector.tensor_tensor(
                                        prd[:],
                                        v[:, 0, ci * CH + h * GW:
                                          ci * CH + (h + 1) * GW],
                                        wreps[:], Op.mult)
                                    for u in range(NSUB):
                                        MM(outp[64 * ci:64 * ci + 64,
                                                h * GW + u * SPL:
                                                h * GW + (u + 1) * SPL],
                                           regsb[y][:, k * 64:(k + 1) * 64],
                                           prd[:, u * SPL:(u + 1) * SPL],
                                           start=(k == 0 and y == 0),
                                           stop=(k == K2 - 1 and y == 1),
                                           skip_group_check=True)
                    for ci in range(2):
                        cb = 2 * hh + ci
                        c8 = g * 4 + cb
                        outs = pc.tile([64, CH], F32, tag="outs", bufs=2)
                        # out col m = p*S + q <- outp col n = q*16 + p
                        opv = outp[64 * ci:64 * ci + 64, :].rearrange(
                            "p (q a) -> p a q", a=16)       # [64, 16, S]
                        nc.scalar.copy(
                            outs[:].rearrange("p (a q) -> p a q", a=16), opv)
                        nc.sync.dma_start(out.ap()[:, c8 * CH:(c8 + 1) * CH],
                                          outs[:])
        es.close()

    nc.compile()
    return nc


# ======================= host-side preparation =======================

def _bf16(x):
    x = np.asarray(x, np.float32)
    u = x.view(np.uint32)
    r = ((u >> 16) + ((u >> 15) & 1)).astype(np.uint16)  # rne-ish
    return r


def _host_consts(w_fuse, w_off, b_off, w_mod, b_mod, w_reg, H=128):
    NPX, NCH, CH, RPC, S, PADH, NTOK, NSPL, SPL = _shape_consts(H)
    import ml_dtypes
    bf = lambda x: np.asarray(x, np.float32).astype(ml_dtypes.bfloat16)

    consts = {}
    wf = np.asarray(w_fuse, np.float32).reshape(64, 128)
    consts["fuse_lhsT"] = bf(np.ascontiguousarray(wf.T))

    w_off = np.asarray(w_off, np.float32).reshape(18, 64, 3, 3)
    w_mod = np.asarray(w_mod, np.float32).reshape(9, 64, 3, 3)

    def qw(q, k):
        return (w_off[2 * k] if q == "dy"
                else w_off[2 * k + 1] if q == "dx" else w_mod[k])

    for q in ("dy", "dx", "mod"):
        for ty in range(3):
            P = np.zeros((128, 32), np.float32)
            Sg = np.zeros((64, 32), np.float32)
            for m in range(18):
                k = m % 9
                P[0:64, m] = qw(q, k)[:, ty, 0]
                P[64:128, m] = qw(q, k)[:, ty, 2]
                Sg[0:64, m] = qw(q, k)[:, ty, 1]
            consts[f"pw_{q}_{ty}"] = bf(P)
            consts[f"sw_{q}_{ty}"] = bf(Sg)

    b_off = np.asarray(b_off, np.float32)
    b_mod = np.asarray(b_mod, np.float32)
    bdy = np.zeros((128, 1), np.float32)
    bdx = np.zeros((128, 1), np.float32)
    bmd = np.zeros((128, 1), np.float32)
    s1 = np.zeros((128, 1), np.float32)
    s2 = np.zeros((128, 1), np.float32)
    for r in range(128):
        rr = r % 32
        if rr < 18:
            k = rr % 9
            bdy[r] = b_off[2 * k]
            bdx[r] = b_off[2 * k + 1]
            bmd[r] = b_mod[k]
        if rr < 9:
            s1[r], s2[r] = -1.0, 1.0
        elif rr < 18:
            s1[r], s2[r] = 1.0, 0.0
    consts["bias_dy"], consts["bias_dx"], consts["bias_mod"] = bdy, bdx, bmd
    consts["xw_s1"], consts["xw_s2"] = s1, s2

    btok = np.zeros((128, 2 * CH), np.float32)
    for r in range(128):
        cb = r // 32
        rr = r % 32
        k = rr % 9 if rr < 18 else 0
        ky, kx = k // 3, k % 3
        for g in range(2):
            c8 = g * 4 + cb
            cols = np.arange(CH)
            px = c8 * CH + cols
            i, j = px // W, px % W
            btok[r, g * CH:(g + 1) * CH] = (i + 1 + ky) * PADW + (j + 1 + kx)
    consts["basetok"] = btok

    selm = np.zeros((128, K2 * 128), np.float32)
    for cb in range(4):
        for k in range(K2):
            selm[32 * cb + k, k * 128:k * 128 + 64] = 1.0
            selm[32 * cb + k + 9, k * 128 + 64:k * 128 + 128] = 1.0
    consts["sel"] = bf(selm)

    w_reg = np.asarray(w_reg, np.float32).reshape(64, 64, 3, 3)
    r0 = np.zeros((128, K2 * 64), np.float32)
    r1 = np.zeros((128, K2 * 64), np.float32)
    for k in range(K2):
        ky, kx = k // 3, k % 3
        blkT = w_reg[:, :, ky, kx].T       # [c, o]
        r0[0:64, k * 64:(k + 1) * 64] = -2.0 * blkT
        r0[64:128, k * 64:(k + 1) * 64] = -2.0 * blkT
        r1[0:64, k * 64:(k + 1) * 64] = 2.0 * blkT
        r1[64:128, k * 64:(k + 1) * 64] = 2.0 * blkT
    consts["reg0"] = bf(r0)
    consts["reg1"] = bf(r1)
    return consts


_NC_CACHE = {}


def kernel(x_img, x_cont, w_fuse, w_off, b_off, w_mod, b_mod, w_reg):
    from concourse.bass_utils import run_bass_kernel_spmd

    H = 128
    B = int(x_img.shape[0])
    NPX = H * W
    if "nc" not in _NC_CACHE:
        _NC_CACHE["nc"] = build_nc(H=H, num_devices=8)
    nc = _NC_CACHE["nc"]

    consts = _host_consts(w_fuse, w_off, b_off, w_mod, b_mod, w_reg, H=H)
    x_img = np.asarray(x_img, np.float32)
    x_cont = np.asarray(x_cont, np.float32)
    in_maps = []
    for b in range(B):
        m = dict(consts)
        m["x_img"] = np.ascontiguousarray(x_img[b].reshape(C, NPX))
        m["x_cont"] = np.ascontiguousarray(x_cont[b].reshape(C, NPX))
        in_maps.append(m)

    res = run_bass_kernel_spmd(nc, in_maps, core_ids=list(range(B)))
    outs = [np.asarray(res.results[b]["out"], np.float32).reshape(C, H, W)
            for b in range(B)]
    return np.stack(outs)


# revision 19
# speedup vs baseline: 1.1108x; 1.1108x over previous
"""Modulated deformable conv (DCNv2-style) Trainium2 Bass kernel.

Batch data-parallel over 8 NeuronCores (1 batch element per core).

Per-core pipeline:
  1. fuse 1x1 conv (PE)  -> x, kept as CHW padded in SBUF (X2, with a
     col-shifted duplicate on partitions 64-127 for K-stacked conv taps)
     and as NHWC 2-pixel tokens in HBM (xtok) for gathering.
  2. dy/dx/mod 3x3 convs (PE, 3 pair-slots K=128 + 3 single-slots K=64).
  3. map pipeline (DVE/ACT): floor flags, fracs, modulated corner-weight
     maps CY0/CY1, int16 gather token indices.
  4. dma_gather (transpose=True, 256B tokens = 2px x 64ch bf16): for each
     kernel tap k and corner row y, V[128=(2px,64ch), npix].
  5. corner weights replicated across 128 partitions via PE selector
     matmuls; prod = V * Wrep (DVE); final contraction on PE with w_reg
     folded lhsT (sign/2x-sigmoid folded in host-side).

Column ordering note: gather list position n maps to map-column
sigma(n) = (n%16)*S + n//16  (S = chunk_cols/16) because indices are
stored 16-partition-wrapped with a contiguous inner dim. sigma is applied
at 3 AP sites: the index wrap DMA, the repl-matmul rhs view, and the
final PSUM->SBUF unpermute copy.
"""
import sys

sys.path.insert(0, "/opt/trn_rl_repo")

from contextlib import ExitStack

import numpy as np

import concourse.bass as bass
import concourse.bacc as bacc
import concourse.mybir as mybir
from concourse.tile import TileContext
from concourse.mybir import AluOpType as Op
from concourse.mybir import ActivationFunctionType as Act

F32 = mybir.dt.float32
BF16 = mybir.dt.bfloat16
I16 = mybir.dt.int16

W = 128
C = 64
K2 = 9
PADW = 132


def _shape_consts(H):
    NPX = H * W
    NCH = 8
    CH = NPX // NCH          # pixels per chunk
    RPC = CH // W            # image rows per chunk
    S = CH // 16             # wrap cols per chunk slot
    PADH = H + 4
    NTOK = PADH * PADW
    NSPL = max(1, CH // 512)
    SPL = CH // NSPL         # matmul N per split (<=512)
    return NPX, NCH, CH, RPC, S, PADH, NTOK, NSPL, SPL


def build_nc(H=128, num_devices=8):
    NPX, NCH, CH, RPC, S, PADH, NTOK, NSPL, SPL = _shape_consts(H)
    RSPL = SPL // W                      # image rows per split
    NTOKP = ((NTOK + 2047) // 2048) * 2048

    es = ExitStack()
    nc = bacc.Bacc("TRN2", target_bir_lowering=False, debug=False,
                   num_devices=num_devices)

    x_img = nc.dram_tensor("x_img", [C, NPX], F32, kind="ExternalInput")
    x_cont = nc.dram_tensor("x_cont", [C, NPX], F32, kind="ExternalInput")
    out = nc.dram_tensor("out", [C, NPX], F32, kind="ExternalOutput")

    fuse_lhsT = nc.dram_tensor("fuse_lhsT", [128, 64], BF16, kind="ExternalInput")
    pw, sw = {}, {}
    for q in ("dy", "dx", "mod"):
        for ty in range(3):
            pw[(q, ty)] = nc.dram_tensor(f"pw_{q}_{ty}", [128, 32], BF16,
                                         kind="ExternalInput")
            sw[(q, ty)] = nc.dram_tensor(f"sw_{q}_{ty}", [64, 32], BF16,
                                         kind="ExternalInput")
    dcols = {}
    for nm in ("bias_dy", "bias_dx", "bias_mod", "xw_s1", "xw_s2"):
        dcols[nm] = nc.dram_tensor(nm, [128, 1], F32, kind="ExternalInput")
    basetok = nc.dram_tensor("basetok", [128, 2 * CH], F32, kind="ExternalInput")
    sel = nc.dram_tensor("sel", [128, K2 * 128], BF16, kind="ExternalInput")
    reg0 = nc.dram_tensor("reg0", [128, K2 * 64], BF16, kind="ExternalInput")
    reg1 = nc.dram_tensor("reg1", [128, K2 * 64], BF16, kind="ExternalInput")

    xtok = nc.dram_tensor("xtok", [NTOKP, 128], BF16, kind="Internal")
    xtok3 = xtok.ap()[0:NTOK, :].rearrange("(a b) e -> a b e", b=PADW)

    MM = lambda *a, **k: nc.tensor.matmul(*a, **k)

    with TileContext(nc) as tc:
        pconst = es.enter_context(tc.tile_pool(name="pconst", bufs=1))
        pp = es.enter_context(tc.tile_pool(name="pp", bufs=1))
        pv = es.enter_context(tc.tile_pool(name="pv", bufs=1))

        # ---- stage constants
        fuse_w = pconst.tile([128, 64], BF16)
        nc.sync.dma_start(fuse_w[:], fuse_lhsT.ap())
        conv_w = {}
        for q in ("dy", "dx", "mod"):
            for ty in range(3):
                tP = pconst.tile([128, 32], BF16, name=f"cwp_{q}{ty}")
                nc.sync.dma_start(tP[:], pw[(q, ty)].ap())
                tS = pconst.tile([64, 32], BF16, name=f"cws_{q}{ty}")
                nc.sync.dma_start(tS[:], sw[(q, ty)].ap())
                conv_w[(q, ty)] = (tP, tS)
        col = {}
        for nm in ("bias_dy", "bias_dx", "bias_mod", "xw_s1", "xw_s2"):
            t = pconst.tile([128, 1], F32, name=f"c_{nm}")
            nc.sync.dma_start(t[:], dcols[nm].ap())
            col[nm] = t
        btok = pconst.tile([128, 2 * CH], F32)
        nc.sync.dma_start(btok[:], basetok.ap())
        sel_sb = pconst.tile([128, K2 * 128], BF16)
        nc.sync.dma_start(sel_sb[:], sel.ap())
        regsb = {}
        for y, t in ((0, reg0), (1, reg1)):
            r = pconst.tile([128, K2 * 64], BF16, name=f"regsb{y}")
            nc.sync.dma_start(r[:], t.ap())
            regsb[y] = r

        CY, IDXT = {}, {}
        WIDX = {y: pp.tile([128, K2 * 8 * S], I16, name=f"widx{y}")
                for y in (0, 1)}

        with tc.tile_pool(name="pX", bufs=1) as pX:
            X2 = pp.tile([128, PADH, PADW], BF16, name="X2")

            # =============== phase 0 ===============
            with tc.tile_pool(name="pin", bufs=1) as pin, \
                 tc.tile_pool(name="p0ps", bufs=2, space="PSUM") as p0ps:
                instk = pin.tile([128, NPX], BF16)
                nc.gpsimd.dma_start(instk[0:64, :], x_img.ap())
                nc.gpsimd.dma_start(instk[64:128, :], x_cont.ap())

                nc.vector.memset(X2[:, :, :], 0.0)
                zt = pin.tile([128, 2048], BF16)
                nc.vector.memset(zt[:, :], 0.0)
                for r0 in range(0, NTOKP, 2048):
                    nc.sync.dma_start(xtok.ap()[r0:r0 + 2048, :], zt[:, :])

                # fuse conv -> X2 rows 0-63 interior
                for c8 in range(NCH):
                    for j in range(NSPL):
                        ps = p0ps.tile([64, SPL], F32, tag="fuseps")
                        off = c8 * CH + j * SPL
                        MM(ps[:], fuse_w[:, :], instk[:, off:off + SPL],
                           start=True, stop=True)
                        i0 = off // W
                        nc.scalar.copy(X2[0:64, 2 + i0:2 + i0 + RSPL, 2:130],
                                       ps[:].rearrange("p (a b) -> p a b", b=W))

                # transposed fuse -> xtok tokens
                stg = pin.tile([128, RPC * 64], BF16, tag="stg", bufs=2)
                for c8 in range(NCH):
                    for r in range(RPC):
                        i = c8 * RPC + r
                        pst = p0ps.tile([128, 64], F32, tag="fuseT")
                        MM(pst[:], instk[:, i * W:(i + 1) * W], fuse_w[:, :],
                           start=True, stop=True)
                        nc.vector.tensor_copy(stg[:, r * 64:(r + 1) * 64], pst[:])
                    rr = c8 * RPC + 2
                    # first halves: token (y, x=2+j)[0:64] = pixel (y, 2+j)
                    nc.sync.dma_start(
                        xtok3[rr:rr + RPC, 2:130, 0:64].transpose([1, 0, 2]),
                        stg[:, :].rearrange("p (r e) -> p r e", e=64))
                    # second halves: token (y, x=1+j)[64:128] = pixel (y, 2+j)
                    nc.sync.dma_start(
                        xtok3[rr:rr + RPC, 1:129, 64:128].transpose([1, 0, 2]),
                        stg[:, :].rearrange("p (r e) -> p r e", e=64))

                # duplicate col-shifted copy on partitions 64-127 (per row-band
                # so convs can start before the whole fuse completes; the 2
                # skipped trailing elems per band are pad zeros on both sides)
                X2f = X2.rearrange("p a b -> p (a b)")
                band = [0] + [2 + c8 * RPC for c8 in range(1, NCH)] + [PADH]
                for bi in range(len(band) - 1):
                    r0, r1 = band[bi], band[bi + 1]
                    n = (r1 - r0) * PADW - 2
                    nc.sync.dma_start(X2f[64:128, r0 * PADW:r0 * PADW + n],
                                      X2f[0:64, r0 * PADW + 2:r0 * PADW + 2 + n])

            # =============== phase A: convs + maps ===============
            with tc.tile_pool(name="paps", bufs=2, space="PSUM") as paps, \
                 tc.tile_pool(name="pam", bufs=1) as pam:
                for g in range(2):
                    qsb = {}
                    for q in ("dy", "dx", "mod"):
                        qps = paps.tile([128, CH], F32, tag="convps")
                        for cb in range(4):
                            c8 = g * 4 + cb
                            for j in range(NSPL):
                                ist = c8 * RPC + j * RSPL
                                dst = qps[32 * cb:32 * cb + 32,
                                          j * SPL:(j + 1) * SPL]
                                for ty in range(3):
                                    tP, tS = conv_w[(q, ty)]
                                    MM(dst,
                                       tP[:, :],
                                       X2[0:128, 1 + ist + ty:1 + ist + ty + RSPL,
                                          1:1 + W],
                                       start=(ty == 0), stop=False,
                                       tile_position=(0, 32 * cb))
                                    MM(dst,
                                       tS[:, :],
                                       X2[0:64, 1 + ist + ty:1 + ist + ty + RSPL,
                                          2:2 + W],
                                       start=False, stop=(ty == 2),
                                       tile_position=(0, 32 * cb))
                        qs = pam.tile([128, CH], BF16, tag=f"q_{q}",
                                      name=f"qsb_{q}{g}")
                        if q == "mod":
                            nc.scalar.activation(qs[:], qps[:], Act.Sigmoid,
                                                 bias=col["bias_mod"][:], scale=1.0)
                        else:
                            nc.scalar.activation(
                                qs[:], qps[:], Act.Identity,
                                bias=col["bias_dy" if q == "dy" else "bias_dx"][:],
                                scale=1.0)
                        qsb[q] = qs

                    FY = pam.tile([128, CH], BF16, tag="m1")
                    nc.vector.tensor_scalar(FY[:], qsb["dy"][:], 0.0, None, Op.is_lt)
                    FX = pam.tile([128, CH], BF16, tag="m2")
                    nc.vector.tensor_scalar(FX[:], qsb["dx"][:], 0.0, None, Op.is_lt)
                    RY = pam.tile([128, CH], BF16, tag="m3")
                    nc.vector.tensor_tensor(RY[:], qsb["dy"][:], FY[:], Op.add)
                    RX = pam.tile([128, CH], BF16, tag="m4")
                    nc.vector.tensor_tensor(RX[:], qsb["dx"][:], FX[:], Op.add)
                    XW = pam.tile([128, CH], BF16, tag="m5")
                    nc.vector.tensor_scalar(XW[:], RX[:], col["xw_s1"][:],
                                            col["xw_s2"][:], Op.mult, Op.add)
                    WY0N = pam.tile([128, CH], BF16, tag="m6")
                    nc.vector.scalar_tensor_tensor(WY0N[:], RY[:], 1.0,
                                                   qsb["mod"][:],
                                                   Op.subtract, Op.mult)
                    RYM = pam.tile([128, CH], BF16, tag="m7")
                    nc.vector.tensor_tensor(RYM[:], RY[:], qsb["mod"][:], Op.mult)
                    cy0 = pp.tile([128, CH], BF16, name=f"cy0_{g}")
                    nc.vector.tensor_tensor(cy0[:], WY0N[:], XW[:], Op.mult)
                    cy1 = pp.tile([128, CH], BF16, name=f"cy1_{g}")
                    nc.vector.tensor_tensor(cy1[:], RYM[:], XW[:], Op.mult)
                    CY[(g, 0)], CY[(g, 1)] = cy0, cy1

                    T1 = pam.tile([128, CH], F32, tag="m8")
                    nc.vector.scalar_tensor_tensor(T1[:], FY[:], -132.0, FX[:],
                                                   Op.mult, Op.subtract)
                    TOK0 = pam.tile([128, CH], F32, tag="m9")
                    nc.vector.tensor_tensor(TOK0[:], btok[:, g * CH:(g + 1) * CH],
                                            T1[:], Op.add)
                    idx0 = pp.tile([128, CH], I16, name=f"idx0_{g}")
                    nc.vector.tensor_copy(idx0[:], TOK0[:])
                    idx1 = pp.tile([128, CH], I16, name=f"idx1_{g}")
                    nc.vector.tensor_scalar(idx1[:], TOK0[:], 132.0, None, Op.add)
                    IDXT[(g, 0)], IDXT[(g, 1)] = idx0, idx1

                # wrapped indices: WIDX[y][p, slot*S + s] = IDX[row, p*S + s]
                for y in (0, 1):
                    for k in range(K2):
                        for g in range(2):
                            for cb in range(4):
                                slot = ((g * K2 + k) * 4 + cb) * S
                                sap = IDXT[(g, y)][32 * cb + k:32 * cb + k + 1, :]
                                eng = nc.sync if (k + cb) % 2 == 0 else nc.scalar
                                eng.dma_start(
                                    WIDX[y][0:16, slot:slot + S],
                                    sap.rearrange("p (a b) -> p a b", b=S))
                    HW_ = K2 * 4 * S
                    for g in range(2):
                        for r8 in range(1, 8):
                            nc.sync.dma_start(
                                WIDX[y][16 * r8:16 * r8 + 16,
                                        g * HW_:(g + 1) * HW_],
                                WIDX[y][0:16, g * HW_:(g + 1) * HW_])

        # =============== phase C: gather / weight / contract ===============
        import os as _os
        if _os.environ.get("SKIP_C"):
            es.close()
            nc.compile()
            return nc
        with tc.tile_pool(name="pcps", bufs=2, space="PSUM") as pcps, \
             tc.tile_pool(name="pops", bufs=1, space="PSUM") as pops, \
             tc.tile_pool(name="pc", bufs=3) as pc:
            for g in range(2):
                for hh in range(2):          # half-group: chunks (2hh, 2hh+1)
                    outp = pops.tile([128, CH], F32, tag="outp", bufs=1)
                    pend, CLAG = [], 3
                    for k in range(K2):
                        for y in (0, 1):
                            v = pv.tile([128, 1, 2 * CH], BF16, tag="vt", bufs=3)
                            islot = ((g * K2 + k) * 4 + 2 * hh) * S
                            nc.gpsimd.dma_gather(
                                v[:, :, :], xtok.ap(),
                                WIDX[y][:, islot:islot + 2 * S],
                                num_idxs=2 * CH, num_idxs_reg=2 * CH,
                                elem_size=128, transpose=True,
                                single_packet=False)
                            NH = NSPL                # 512-col groups
                            GW = CH // NH            # cols per group
                            NSUB = GW // SPL         # matmul splits per group
                            for ci in range(2):
                                cb = 2 * hh + ci
                                cy = CY[(g, y)]
                                cyv = cy[32 * cb:32 * cb + 18, :].rearrange(
                                    "p (a b) -> p b a", b=S)   # [18, S, 16]
                                for h in range(NH):
                                    wrepp = pcps.tile([128, GW], F32, tag="wrepp", bufs=4)
                                    for u in range(NSUB):
                                        q0 = (h * GW + u * SPL) // 16
                                        MM(wrepp[:, u * SPL:(u + 1) * SPL],
                                           sel_sb[32 * cb:32 * cb + 18,
                                                  k * 128:(k + 1) * 128],
                                           cyv[:, q0:q0 + SPL // 16, :],
                                           start=True, stop=True,
                                           tile_position=(32 * cb, 0),
                                           skip_group_check=True)
                                    wreps = pc.tile([128, GW], BF16, tag="wreps",
                                                    bufs=8)
                                    if (k + y + h) % 3 < 2:
                                        nc.scalar.copy(wreps[:], wrepp[:])
                                    else:
                                        nc.vector.tensor_copy(wreps[:], wrepp[:])
                                    prd = pc.tile([128, GW], BF16, tag="prd",
                                                  bufs=8)
                                    nc.vector.tensor_tensor(
                                        prd[:],
                                        v[:, 0, ci * CH + h * GW:
                                          ci * CH + (h + 1) * GW],
                                        wreps[:], Op.mult)

                                    def _emit_contr(prd=prd, y=y, k=k, ci=ci,
                                                    h=h, outp=outp, GW=GW):
                                        for u in range(NSUB):
                                            MM(outp[64 * ci:64 * ci + 64,
                                                    h * GW + u * SPL:
                                                    h * GW + (u + 1) * SPL],
                                               regsb[y][:, k * 64:(k + 1) * 64],
                                               prd[:, u * SPL:(u + 1) * SPL],
                                               start=(k == 0 and y == 0),
                                               stop=(k == K2 - 1 and y == 1),
                                               skip_group_check=True)
                                    pend.append(_emit_contr)
                                    if len(pend) > CLAG:
                                        pend.pop(0)()
                    for fe in pend:
                        fe()
                    for ci in range(2):
                        cb = 2 * hh + ci
                        c8 = g * 4 + cb
                        outs = pc.tile([64, CH], F32, tag="outs", bufs=2)
                        # out col m = p*S + q <- outp col n = q*16 + p
                        opv = outp[64 * ci:64 * ci + 64, :].rearrange(
                            "p (q a) -> p a q", a=16)       # [64, 16, S]
                        nc.scalar.copy(
                            outs[:].rearrange("p (a q) -> p a q", a=16), opv)
                        nc.sync.dma_start(out.ap()[:, c8 * CH:(c8 + 1) * CH],
                                          outs[:])
        es.close()

    nc.compile()
    return nc


# ======================= host-side preparation =======================

def _bf16(x):
    x = np.asarray(x, np.float32)
    u = x.view(np.uint32)
    r = ((u >> 16) + ((u >> 15) & 1)).astype(np.uint16)  # rne-ish
    return r


def _host_consts(w_fuse, w_off, b_off, w_mod, b_mod, w_reg, H=128):
    NPX, NCH, CH, RPC, S, PADH, NTOK, NSPL, SPL = _shape_consts(H)
    import ml_dtypes
    bf = lambda x: np.asarray(x, np.float32).astype(ml_dtypes.bfloat16)

    consts = {}
    wf = np.asarray(w_fuse, np.float32).reshape(64, 128)
    consts["fuse_lhsT"] = bf(np.ascontiguousarray(wf.T))

    w_off = np.asarray(w_off, np.float32).reshape(18, 64, 3, 3)
    w_mod = np.asarray(w_mod, np.float32).reshape(9, 64, 3, 3)

    def qw(q, k):
        return (w_off[2 * k] if q == "dy"
                else w_off[2 * k + 1] if q == "dx" else w_mod[k])

    for q in ("dy", "dx", "mod"):
        for ty in range(3):
            P = np.zeros((128, 32), np.float32)
            Sg = np.zeros((64, 32), np.float32)
            for m in range(18):
                k = m % 9
                P[0:64, m] = qw(q, k)[:, ty, 0]
                P[64:128, m] = qw(q, k)[:, ty, 2]
                Sg[0:64, m] = qw(q, k)[:, ty, 1]
            consts[f"pw_{q}_{ty}"] = bf(P)
            consts[f"sw_{q}_{ty}"] = bf(Sg)

    b_off = np.asarray(b_off, np.float32)
    b_mod = np.asarray(b_mod, np.float32)
    bdy = np.zeros((128, 1), np.float32)
    bdx = np.zeros((128, 1), np.float32)
    bmd = np.zeros((128, 1), np.float32)
    s1 = np.zeros((128, 1), np.float32)
    s2 = np.zeros((128, 1), np.float32)
    for r in range(128):
        rr = r % 32
        if rr < 18:
            k = rr % 9
            bdy[r] = b_off[2 * k]
            bdx[r] = b_off[2 * k + 1]
            bmd[r] = b_mod[k]
        if rr < 9:
            s1[r], s2[r] = -1.0, 1.0
        elif rr < 18:
            s1[r], s2[r] = 1.0, 0.0
    consts["bias_dy"], consts["bias_dx"], consts["bias_mod"] = bdy, bdx, bmd
    consts["xw_s1"], consts["xw_s2"] = s1, s2

    btok = np.zeros((128, 2 * CH), np.float32)
    for r in range(128):
        cb = r // 32
        rr = r % 32
        k = rr % 9 if rr < 18 else 0
        ky, kx = k // 3, k % 3
        for g in range(2):
            c8 = g * 4 + cb
            cols = np.arange(CH)
            px = c8 * CH + cols
            i, j = px // W, px % W
            btok[r, g * CH:(g + 1) * CH] = (i + 1 + ky) * PADW + (j + 1 + kx)
    consts["basetok"] = btok

    selm = np.zeros((128, K2 * 128), np.float32)
    for cb in range(4):
        for k in range(K2):
            selm[32 * cb + k, k * 128:k * 128 + 64] = 1.0
            selm[32 * cb + k + 9, k * 128 + 64:k * 128 + 128] = 1.0
    consts["sel"] = bf(selm)

    w_reg = np.asarray(w_reg, np.float32).reshape(64, 64, 3, 3)
    r0 = np.zeros((128, K2 * 64), np.float32)
    r1 = np.zeros((128, K2 * 64), np.float32)
    for k in range(K2):
        ky, kx = k // 3, k % 3
        blkT = w_reg[:, :, ky, kx].T       # [c, o]
        r0[0:64, k * 64:(k + 1) * 64] = -2.0 * blkT
        r0[64:128, k * 64:(k + 1) * 64] = -2.0 * blkT
        r1[0:64, k * 64:(k + 1) * 64] = 2.0 * blkT
        r1[64:128, k * 64:(k + 1) * 64] = 2.0 * blkT
    consts["reg0"] = bf(r0)
    consts["reg1"] = bf(r1)
    return consts


_NC_CACHE = {}


def kernel(x_img, x_cont, w_fuse, w_off, b_off, w_mod, b_mod, w_reg):
    from concourse.bass_utils import run_bass_kernel_spmd

    H = 128
    B = int(x_img.shape[0])
    NPX = H * W
    if "nc" not in _NC_CACHE:
        _NC_CACHE["nc"] = build_nc(H=H, num_devices=8)
    nc = _NC_CACHE["nc"]

    consts = _host_consts(w_fuse, w_off, b_off, w_mod, b_mod, w_reg, H=H)
    x_img = np.asarray(x_img, np.float32)
    x_cont = np.asarray(x_cont, np.float32)
    in_maps = []
    for b in range(B):
        m = dict(consts)
        m["x_img"] = np.ascontiguousarray(x_img[b].reshape(C, NPX))
        m["x_cont"] = np.ascontiguousarray(x_cont[b].reshape(C, NPX))
        in_maps.append(m)

    res = run_bass_kernel_spmd(nc, in_maps, core_ids=list(range(B)))
    outs = [np.asarray(res.results[b]["out"], np.float32).reshape(C, H, W)
            for b in range(B)]
    return np.stack(outs)


# revision 23
# speedup vs baseline: 1.3096x; 1.1790x over previous
"""Modulated deformable conv (DCNv2-style) Trainium2 Bass kernel.

Batch data-parallel over 8 NeuronCores (1 batch element per core).

Per-core pipeline:
  1. fuse 1x1 conv (PE)  -> x, kept as CHW padded in SBUF (X2, with a
     col-shifted duplicate on partitions 64-127 for K-stacked conv taps)
     and as NHWC 2-pixel tokens in HBM (xtok) for gathering.
  2. dy/dx/mod 3x3 convs (PE, 3 pair-slots K=128 + 3 single-slots K=64).
  3. map pipeline (DVE/ACT): floor flags, fracs, modulated corner-weight
     maps CY0/CY1, int16 gather token indices.
  4. dma_gather (transpose=True, 256B tokens = 2px x 64ch bf16): for each
     kernel tap k and corner row y, V[128=(2px,64ch), npix].
  5. corner weights replicated across 128 partitions via PE selector
     matmuls; prod = V * Wrep (DVE); final contraction on PE with w_reg
     folded lhsT (sign/2x-sigmoid folded in host-side).

Column ordering note: gather list position n maps to map-column
sigma(n) = (n%16)*S + n//16  (S = chunk_cols/16) because indices are
stored 16-partition-wrapped with a contiguous inner dim. sigma is applied
at 3 AP sites: the index wrap DMA, the repl-matmul rhs view, and the
final PSUM->SBUF unpermute copy.
"""
import sys

sys.path.insert(0, "/opt/trn_rl_repo")

from contextlib import ExitStack

import numpy as np

import concourse.bass as bass
import concourse.bacc as bacc
import concourse.mybir as mybir
from concourse.tile import TileContext
from concourse.mybir import AluOpType as Op
from concourse.mybir import ActivationFunctionType as Act

F32 = mybir.dt.float32
BF16 = mybir.dt.bfloat16
I16 = mybir.dt.int16

W = 128
C = 64
K2 = 9
PADW = 132


def _shape_consts(H):
    NPX = H * W
    NCH = 8
    CH = NPX // NCH          # pixels per chunk
    RPC = CH // W            # image rows per chunk
    S = CH // 16             # wrap cols per chunk slot
    PADH = H + 4
    NTOK = PADH * PADW
    NSPL = max(1, CH // 512)
    SPL = CH // NSPL         # matmul N per split (<=512)
    return NPX, NCH, CH, RPC, S, PADH, NTOK, NSPL, SPL


def build_nc(H=128, num_devices=8):
    NPX, NCH, CH, RPC, S, PADH, NTOK, NSPL, SPL = _shape_consts(H)
    RSPL = SPL // W                      # image rows per split
    NTOKP = ((NTOK + 2047) // 2048) * 2048

    es = ExitStack()
    nc = bacc.Bacc("TRN2", target_bir_lowering=False, debug=False,
                   num_devices=num_devices)

    x_img = nc.dram_tensor("x_img", [C, NPX], F32, kind="ExternalInput")
    x_cont = nc.dram_tensor("x_cont", [C, NPX], F32, kind="ExternalInput")
    out = nc.dram_tensor("out", [C, NPX], F32, kind="ExternalOutput")

    fuse_lhsT = nc.dram_tensor("fuse_lhsT", [128, 64], BF16, kind="ExternalInput")
    pw, sw = {}, {}
    for q in ("dy", "dx", "mod"):
        for ty in range(3):
            pw[(q, ty)] = nc.dram_tensor(f"pw_{q}_{ty}", [128, 32], BF16,
                                         kind="ExternalInput")
            sw[(q, ty)] = nc.dram_tensor(f"sw_{q}_{ty}", [64, 32], BF16,
                                         kind="ExternalInput")
    dcols = {}
    for nm in ("bias_dy", "bias_dx", "bias_mod", "xw_s1", "xw_s2"):
        dcols[nm] = nc.dram_tensor(nm, [128, 1], F32, kind="ExternalInput")
    bt0 = nc.dram_tensor("bt0", [128, 2 * CH], F32, kind="ExternalInput")
    btd = nc.dram_tensor("btd", [128, 2 * CH], F32, kind="ExternalInput")
    sel = nc.dram_tensor("sel", [128, K2 * 128], BF16, kind="ExternalInput")
    reg0 = nc.dram_tensor("reg0", [128, K2 * 64], BF16, kind="ExternalInput")
    reg1 = nc.dram_tensor("reg1", [128, K2 * 64], BF16, kind="ExternalInput")

    xtok = nc.dram_tensor("xtok", [NTOKP, 128], BF16, kind="Internal")
    xtok3 = xtok.ap()[0:NTOK, :].rearrange("(a b) e -> a b e", b=PADW)
    NPAIR_P = PADH // 2
    NPAIR_Q = PADH // 2 - 1
    QBASE = NPAIR_P * PADW
    NTOK2 = (NPAIR_P + NPAIR_Q) * PADW
    xtok2 = nc.dram_tensor("xtok2", [NTOK2, 256], BF16, kind="Internal")

    MM = lambda *a, **k: nc.tensor.matmul(*a, **k)

    with TileContext(nc) as tc:
        pconst = es.enter_context(tc.tile_pool(name="pconst", bufs=1))
        pp = es.enter_context(tc.tile_pool(name="pp", bufs=1))
        pv = es.enter_context(tc.tile_pool(name="pv", bufs=1))

        # ---- stage constants
        fuse_w = pconst.tile([128, 64], BF16)
        nc.sync.dma_start(fuse_w[:], fuse_lhsT.ap())
        conv_w = {}
        for q in ("dy", "dx", "mod"):
            for ty in range(3):
                tP = pconst.tile([128, 32], BF16, name=f"cwp_{q}{ty}")
                nc.sync.dma_start(tP[:], pw[(q, ty)].ap())
                tS = pconst.tile([64, 32], BF16, name=f"cws_{q}{ty}")
                nc.sync.dma_start(tS[:], sw[(q, ty)].ap())
                conv_w[(q, ty)] = (tP, tS)
        col = {}
        for nm in ("bias_dy", "bias_dx", "bias_mod", "xw_s1", "xw_s2"):
            t = pconst.tile([128, 1], F32, name=f"c_{nm}")
            nc.sync.dma_start(t[:], dcols[nm].ap())
            col[nm] = t
        btok0 = pconst.tile([128, 2 * CH], F32)
        nc.sync.dma_start(btok0[:], bt0.ap())
        btokd = pconst.tile([128, 2 * CH], F32)
        nc.sync.dma_start(btokd[:], btd.ap())
        sel_sb = pconst.tile([128, K2 * 128], BF16)
        nc.sync.dma_start(sel_sb[:], sel.ap())
        regsb = {}
        for y, t in ((0, reg0), (1, reg1)):
            r = pconst.tile([128, K2 * 64], BF16, name=f"regsb{y}")
            nc.sync.dma_start(r[:], t.ap())
            regsb[y] = r

        CY, IDXT = {}, {}
        WIDX = pp.tile([128, K2 * 8 * S], I16, name="widx")

        with tc.tile_pool(name="pX", bufs=1) as pX:
            X2 = pp.tile([128, PADH, PADW], BF16, name="X2")

            # =============== phase 0 ===============
            with tc.tile_pool(name="pin", bufs=1) as pin, \
                 tc.tile_pool(name="p0ps", bufs=2, space="PSUM") as p0ps:
                instk = pin.tile([128, NPX], BF16)
                nc.gpsimd.dma_start(instk[0:64, :], x_img.ap())
                nc.gpsimd.dma_start(instk[64:128, :], x_cont.ap())

                nc.vector.memset(X2[:, :, :], 0.0)
                zt = pin.tile([128, 2048], BF16)
                nc.vector.memset(zt[:, :], 0.0)
                for r0 in range(0, NTOKP, 2048):
                    nc.sync.dma_start(xtok.ap()[r0:r0 + 2048, :], zt[:, :])

                # fuse conv -> X2 rows 0-63 interior
                for c8 in range(NCH):
                    for j in range(NSPL):
                        ps = p0ps.tile([64, SPL], F32, tag="fuseps")
                        off = c8 * CH + j * SPL
                        MM(ps[:], fuse_w[:, :], instk[:, off:off + SPL],
                           start=True, stop=True)
                        i0 = off // W
                        nc.scalar.copy(X2[0:64, 2 + i0:2 + i0 + RSPL, 2:130],
                                       ps[:].rearrange("p (a b) -> p a b", b=W))

                # transposed fuse -> xtok tokens
                stg = pin.tile([128, RPC * 64], BF16, tag="stg", bufs=2)
                for c8 in range(NCH):
                    for r in range(RPC):
                        i = c8 * RPC + r
                        pst = p0ps.tile([128, 64], F32, tag="fuseT")
                        MM(pst[:], instk[:, i * W:(i + 1) * W], fuse_w[:, :],
                           start=True, stop=True)
                        nc.vector.tensor_copy(stg[:, r * 64:(r + 1) * 64], pst[:])
                    rr = c8 * RPC + 2
                    # first halves: token (y, x=2+j)[0:64] = pixel (y, 2+j)
                    nc.sync.dma_start(
                        xtok3[rr:rr + RPC, 2:130, 0:64].transpose([1, 0, 2]),
                        stg[:, :].rearrange("p (r e) -> p r e", e=64))
                    # second halves: token (y, x=1+j)[64:128] = pixel (y, 2+j)
                    nc.sync.dma_start(
                        xtok3[rr:rr + RPC, 1:129, 64:128].transpose([1, 0, 2]),
                        stg[:, :].rearrange("p (r e) -> p r e", e=64))

                # duplicate col-shifted copy on partitions 64-127 (per row-band
                # so convs can start before the whole fuse completes; the 2
                # skipped trailing elems per band are pad zeros on both sides)
                X2f = X2.rearrange("p a b -> p (a b)")
                band = [0] + [2 + c8 * RPC for c8 in range(1, NCH)] + [PADH]
                for bi in range(len(band) - 1):
                    r0, r1 = band[bi], band[bi + 1]
                    n = (r1 - r0) * PADW - 2
                    nc.sync.dma_start(X2f[64:128, r0 * PADW:r0 * PADW + n],
                                      X2f[0:64, r0 * PADW + 2:r0 * PADW + 2 + n])

            # derive 512B pair-row tokens: P copy (even y0), Q copy (odd y0)
            for par, npair, base in ((0, NPAIR_P, 0), (1, NPAIR_Q, QBASE)):
                for half in range(2):
                    nc.sync.dma_start(
                        xtok2.ap()[base:base + npair * PADW,
                                   128 * half:128 * half + 128]
                        .rearrange("(a b) e -> a b e", b=PADW),
                        xtok.ap()[(par + half) * PADW:
                                  (par + half) * PADW + npair * 2 * PADW, :]
                        .rearrange("(a c b) e -> a (c b) e", c=2, b=PADW)
                        [:, 0:PADW, :])

            # =============== phase A: convs + maps ===============
            with tc.tile_pool(name="paps", bufs=2, space="PSUM") as paps, \
                 tc.tile_pool(name="pam", bufs=1) as pam:
                for g in range(2):
                    qsb = {}
                    for q in ("dy", "dx", "mod"):
                        qps = paps.tile([128, CH], F32, tag="convps")
                        for cb in range(4):
                            c8 = g * 4 + cb
                            for j in range(NSPL):
                                ist = c8 * RPC + j * RSPL
                                dst = qps[32 * cb:32 * cb + 32,
                                          j * SPL:(j + 1) * SPL]
                                for ty in range(3):
                                    tP, tS = conv_w[(q, ty)]
                                    MM(dst,
                                       tP[:, :],
                                       X2[0:128, 1 + ist + ty:1 + ist + ty + RSPL,
                                          1:1 + W],
                                       start=(ty == 0), stop=False,
                                       tile_position=(0, 32 * cb))
                                    MM(dst,
                                       tS[:, :],
                                       X2[0:64, 1 + ist + ty:1 + ist + ty + RSPL,
                                          2:2 + W],
                                       start=False, stop=(ty == 2),
                                       tile_position=(0, 32 * cb))
                        qs = pam.tile([128, CH], BF16, tag=f"q_{q}",
                                      name=f"qsb_{q}{g}")
                        if q == "mod":
                            nc.scalar.activation(qs[:], qps[:], Act.Sigmoid,
                                                 bias=col["bias_mod"][:], scale=1.0)
                        else:
                            nc.scalar.activation(
                                qs[:], qps[:], Act.Identity,
                                bias=col["bias_dy" if q == "dy" else "bias_dx"][:],
                                scale=1.0)
                        qsb[q] = qs

                    FY = pam.tile([128, CH], BF16, tag="m1")
                    nc.vector.tensor_scalar(FY[:], qsb["dy"][:], 0.0, None, Op.is_lt)
                    FX = pam.tile([128, CH], BF16, tag="m2")
                    nc.vector.tensor_scalar(FX[:], qsb["dx"][:], 0.0, None, Op.is_lt)
                    RY = pam.tile([128, CH], BF16, tag="m3")
                    nc.vector.tensor_tensor(RY[:], qsb["dy"][:], FY[:], Op.add)
                    RX = pam.tile([128, CH], BF16, tag="m4")
                    nc.vector.tensor_tensor(RX[:], qsb["dx"][:], FX[:], Op.add)
                    XW = pam.tile([128, CH], BF16, tag="m5")
                    nc.vector.tensor_scalar(XW[:], RX[:], col["xw_s1"][:],
                                            col["xw_s2"][:], Op.mult, Op.add)
                    WY0N = pam.tile([128, CH], BF16, tag="m6")
                    nc.vector.scalar_tensor_tensor(WY0N[:], RY[:], 1.0,
                                                   qsb["mod"][:],
                                                   Op.subtract, Op.mult)
                    RYM = pam.tile([128, CH], BF16, tag="m7")
                    nc.vector.tensor_tensor(RYM[:], RY[:], qsb["mod"][:], Op.mult)
                    cy0 = pp.tile([128, CH], BF16, name=f"cy0_{g}")
                    nc.vector.tensor_tensor(cy0[:], WY0N[:], XW[:], Op.mult)
                    cy1 = pp.tile([128, CH], BF16, name=f"cy1_{g}")
                    nc.vector.tensor_tensor(cy1[:], RYM[:], XW[:], Op.mult)
                    CY[(g, 0)], CY[(g, 1)] = cy0, cy1

                    T1 = pam.tile([128, CH], F32, tag="m8")
                    nc.vector.tensor_tensor(T1[:], FY[:],
                                            btokd[:, g * CH:(g + 1) * CH], Op.mult)
                    TOK0 = pam.tile([128, CH], F32, tag="m9")
                    nc.vector.tensor_tensor(TOK0[:], btok0[:, g * CH:(g + 1) * CH],
                                            T1[:], Op.subtract)
                    T2 = pam.tile([128, CH], F32, tag="m8", name="T2")
                    nc.vector.tensor_tensor(T2[:], TOK0[:], FX[:], Op.subtract)
                    idx0 = pp.tile([128, CH], I16, name=f"idx0_{g}")
                    nc.vector.tensor_copy(idx0[:], T2[:])
                    IDXT[g] = idx0

                # wrapped indices: WIDX[y][p, slot*S + s] = IDX[row, p*S + s]
                for k in range(K2):
                    for g in range(2):
                        for cb in range(4):
                            slot = ((g * K2 + k) * 4 + cb) * S
                            sap = IDXT[g][32 * cb + k:32 * cb + k + 1, :]
                            eng = nc.sync if (k + cb) % 2 == 0 else nc.scalar
                            eng.dma_start(
                                WIDX[0:16, slot:slot + S],
                                sap.rearrange("p (a b) -> p a b", b=S))
                HW_ = K2 * 4 * S
                for g in range(2):
                    for r8 in range(1, 8):
                        nc.sync.dma_start(
                            WIDX[16 * r8:16 * r8 + 16, g * HW_:(g + 1) * HW_],
                            WIDX[0:16, g * HW_:(g + 1) * HW_])

        # =============== phase C: gather / weight / contract ===============
        import os as _os
        if _os.environ.get("SKIP_C"):
            es.close()
            nc.compile()
            return nc
        with tc.tile_pool(name="pcps", bufs=2, space="PSUM") as pcps, \
             tc.tile_pool(name="pops", bufs=1, space="PSUM") as pops, \
             tc.tile_pool(name="pc", bufs=3) as pc:
            for g in range(2):
                for hh in range(2):          # half-group: chunks (2hh, 2hh+1)
                    outp = pops.tile([128, CH], F32, tag="outp", bufs=1)
                    pend, CLAG = [], 3
                    for k in range(K2):
                        v = pv.tile([128, 2, 2 * CH], BF16, tag="vt", bufs=2)
                        islot = ((g * K2 + k) * 4 + 2 * hh) * S
                        if not _os.environ.get("NO_GATHER"):
                            nc.gpsimd.dma_gather(
                                v[:, :, :], xtok2.ap(),
                                WIDX[:, islot:islot + 2 * S],
                                num_idxs=2 * CH, num_idxs_reg=2 * CH,
                                elem_size=256, transpose=True,
                                single_packet=False)
                        for y in (0, 1):
                            if _os.environ.get("NO_COMPUTE"):
                                continue
                            NH = NSPL                # 512-col groups
                            GW = CH // NH            # cols per group
                            NSUB = GW // SPL         # matmul splits per group
                            for ci in range(2):
                                cb = 2 * hh + ci
                                cy = CY[(g, y)]
                                cyv = cy[32 * cb:32 * cb + 18, :].rearrange(
                                    "p (a b) -> p b a", b=S)   # [18, S, 16]
                                for h in range(NH):
                                    wrepp = pcps.tile([128, GW], F32, tag="wrepp", bufs=4)
                                    for u in range(NSUB):
                                        q0 = (h * GW + u * SPL) // 16
                                        MM(wrepp[:, u * SPL:(u + 1) * SPL],
                                           sel_sb[32 * cb:32 * cb + 18,
                                                  k * 128:(k + 1) * 128],
                                           cyv[:, q0:q0 + SPL // 16, :],
                                           start=True, stop=True,
                                           tile_position=(32 * cb, 0),
                                           skip_group_check=True)
                                    wreps = pc.tile([128, GW], BF16, tag="wreps",
                                                    bufs=8)
                                    if (k + y + h) % 3 < 2:
                                        nc.scalar.copy(wreps[:], wrepp[:])
                                    else:
                                        nc.vector.tensor_copy(wreps[:], wrepp[:])
                                    prd = pc.tile([128, GW], BF16, tag="prd",
                                                  bufs=8)
                                    nc.vector.tensor_tensor(
                                        prd[:],
                                        v[:, y, ci * CH + h * GW:
                                          ci * CH + (h + 1) * GW],
                                        wreps[:], Op.mult)

                                    def _emit_contr(prd=prd, y=y, k=k, ci=ci,
                                                    h=h, outp=outp, GW=GW):
                                        for u in range(NSUB):
                                            MM(outp[64 * ci:64 * ci + 64,
                                                    h * GW + u * SPL:
                                                    h * GW + (u + 1) * SPL],
                                               regsb[y][:, k * 64:(k + 1) * 64],
                                               prd[:, u * SPL:(u + 1) * SPL],
                                               start=(k == 0 and y == 0),
                                               stop=(k == K2 - 1 and y == 1),
                                               skip_group_check=True)
                                    pend.append(_emit_contr)
                                    if len(pend) > CLAG:
                                        pend.pop(0)()
                    for fe in pend:
                        fe()
                    if _os.environ.get("NO_COMPUTE"):
                        nc.vector.memset(outp[:, :], 0.0)
                    for ci in range(2):
                        cb = 2 * hh + ci
                        c8 = g * 4 + cb
                        outs = pc.tile([64, CH], F32, tag="outs", bufs=2)
                        # out col m = p*S + q <- outp col n = q*16 + p
                        opv = outp[64 * ci:64 * ci + 64, :].rearrange(
                            "p (q a) -> p a q", a=16)       # [64, 16, S]
                        nc.scalar.copy(
                            outs[:].rearrange("p (a q) -> p a q", a=16), opv)
                        nc.sync.dma_start(out.ap()[:, c8 * CH:(c8 + 1) * CH],
                                          outs[:])
        es.close()

    nc.compile()
    return nc


# ======================= host-side preparation =======================

def _bf16(x):
    x = np.asarray(x, np.float32)
    u = x.view(np.uint32)
    r = ((u >> 16) + ((u >> 15) & 1)).astype(np.uint16)  # rne-ish
    return r


def _host_consts(w_fuse, w_off, b_off, w_mod, b_mod, w_reg, H=128):
    NPX, NCH, CH, RPC, S, PADH, NTOK, NSPL, SPL = _shape_consts(H)
    import ml_dtypes
    bf = lambda x: np.asarray(x, np.float32).astype(ml_dtypes.bfloat16)

    consts = {}
    wf = np.asarray(w_fuse, np.float32).reshape(64, 128)
    consts["fuse_lhsT"] = bf(np.ascontiguousarray(wf.T))

    w_off = np.asarray(w_off, np.float32).reshape(18, 64, 3, 3)
    w_mod = np.asarray(w_mod, np.float32).reshape(9, 64, 3, 3)

    def qw(q, k):
        return (w_off[2 * k] if q == "dy"
                else w_off[2 * k + 1] if q == "dx" else w_mod[k])

    for q in ("dy", "dx", "mod"):
        for ty in range(3):
            P = np.zeros((128, 32), np.float32)
            Sg = np.zeros((64, 32), np.float32)
            for m in range(18):
                k = m % 9
                P[0:64, m] = qw(q, k)[:, ty, 0]
                P[64:128, m] = qw(q, k)[:, ty, 2]
                Sg[0:64, m] = qw(q, k)[:, ty, 1]
            consts[f"pw_{q}_{ty}"] = bf(P)
            consts[f"sw_{q}_{ty}"] = bf(Sg)

    b_off = np.asarray(b_off, np.float32)
    b_mod = np.asarray(b_mod, np.float32)
    bdy = np.zeros((128, 1), np.float32)
    bdx = np.zeros((128, 1), np.float32)
    bmd = np.zeros((128, 1), np.float32)
    s1 = np.zeros((128, 1), np.float32)
    s2 = np.zeros((128, 1), np.float32)
    for r in range(128):
        rr = r % 32
        if rr < 18:
            k = rr % 9
            bdy[r] = b_off[2 * k]
            bdx[r] = b_off[2 * k + 1]
            bmd[r] = b_mod[k]
        if rr < 9:
            s1[r], s2[r] = -1.0, 1.0
        elif rr < 18:
            s1[r], s2[r] = 1.0, 0.0
    consts["bias_dy"], consts["bias_dx"], consts["bias_mod"] = bdy, bdx, bmd
    consts["xw_s1"], consts["xw_s2"] = s1, s2

    PADH_ = H + 4
    QBASE = (PADH_ // 2) * PADW

    def _pairtok(y0):
        return np.where(y0 % 2 == 0, (y0 // 2) * PADW,
                        QBASE + (y0 // 2) * PADW)

    b0 = np.zeros((128, 2 * CH), np.float32)
    bd = np.zeros((128, 2 * CH), np.float32)
    for r in range(128):
        cb = r // 32
        rr = r % 32
        k = rr % 9 if rr < 18 else 0
        ky, kx = k // 3, k % 3
        for g in range(2):
            c8 = g * 4 + cb
            cols = np.arange(CH)
            px = c8 * CH + cols
            i, j = px // W, px % W
            yb = i + 1 + ky
            pt0 = _pairtok(yb) + (j + 1 + kx)
            ptm = _pairtok(yb - 1) + (j + 1 + kx)
            b0[r, g * CH:(g + 1) * CH] = pt0
            bd[r, g * CH:(g + 1) * CH] = pt0 - ptm
    consts["bt0"] = b0
    consts["btd"] = bd

    selm = np.zeros((128, K2 * 128), np.float32)
    for cb in range(4):
        for k in range(K2):
            selm[32 * cb + k, k * 128:k * 128 + 64] = 1.0
            selm[32 * cb + k + 9, k * 128 + 64:k * 128 + 128] = 1.0
    consts["sel"] = bf(selm)

    w_reg = np.asarray(w_reg, np.float32).reshape(64, 64, 3, 3)
    r0 = np.zeros((128, K2 * 64), np.float32)
    r1 = np.zeros((128, K2 * 64), np.float32)
    for k in range(K2):
        ky, kx = k // 3, k % 3
        blkT = w_reg[:, :, ky, kx].T       # [c, o]
        r0[0:64, k * 64:(k + 1) * 64] = -2.0 * blkT
        r0[64:128, k * 64:(k + 1) * 64] = -2.0 * blkT
        r1[0:64, k * 64:(k + 1) * 64] = 2.0 * blkT
        r1[64:128, k * 64:(k + 1) * 64] = 2.0 * blkT
    consts["reg0"] = bf(r0)
    consts["reg1"] = bf(r1)
    return consts


_NC_CACHE = {}


def kernel(x_img, x_cont, w_fuse, w_off, b_off, w_mod, b_mod, w_reg):
    from concourse.bass_utils import run_bass_kernel_spmd

    H = 128
    B = int(x_img.shape[0])
    NPX = H * W
    if "nc" not in _NC_CACHE:
        _NC_CACHE["nc"] = build_nc(H=H, num_devices=8)
    nc = _NC_CACHE["nc"]

    consts = _host_consts(w_fuse, w_off, b_off, w_mod, b_mod, w_reg, H=H)
    x_img = np.asarray(x_img, np.float32)
    x_cont = np.asarray(x_cont, np.float32)
    in_maps = []
    for b in range(B):
        m = dict(consts)
        m["x_img"] = np.ascontiguousarray(x_img[b].reshape(C, NPX))
        m["x_cont"] = np.ascontiguousarray(x_cont[b].reshape(C, NPX))
        in_maps.append(m)

    res = run_bass_kernel_spmd(nc, in_maps, core_ids=list(range(B)))
    outs = [np.asarray(res.results[b]["out"], np.float32).reshape(C, H, W)
            for b in range(B)]
    return np.stack(outs)


# revision 27
# speedup vs baseline: 1.3527x; 1.0329x over previous
"""Modulated deformable conv (DCNv2-style) Trainium2 Bass kernel.

Batch data-parallel over 8 NeuronCores (1 batch element per core).

Per-core pipeline:
  1. fuse 1x1 conv (PE)  -> x, kept as CHW padded in SBUF (X2, with a
     col-shifted duplicate on partitions 64-127 for K-stacked conv taps)
     and as NHWC 2-pixel tokens in HBM (xtok) for gathering.
  2. dy/dx/mod 3x3 convs (PE, 3 pair-slots K=128 + 3 single-slots K=64).
  3. map pipeline (DVE/ACT): floor flags, fracs, modulated corner-weight
     maps CY0/CY1, int16 gather token indices.
  4. dma_gather (transpose=True, 256B tokens = 2px x 64ch bf16): for each
     kernel tap k and corner row y, V[128=(2px,64ch), npix].
  5. corner weights replicated across 128 partitions via PE selector
     matmuls; prod = V * Wrep (DVE); final contraction on PE with w_reg
     folded lhsT (sign/2x-sigmoid folded in host-side).

Column ordering note: gather list position n maps to map-column
sigma(n) = (n%16)*S + n//16  (S = chunk_cols/16) because indices are
stored 16-partition-wrapped with a contiguous inner dim. sigma is applied
at 3 AP sites: the index wrap DMA, the repl-matmul rhs view, and the
final PSUM->SBUF unpermute copy.
"""
import sys

sys.path.insert(0, "/opt/trn_rl_repo")

from contextlib import ExitStack

import numpy as np

import concourse.bass as bass
import concourse.bacc as bacc
import concourse.mybir as mybir
from concourse.tile import TileContext
from concourse.mybir import AluOpType as Op
from concourse.mybir import ActivationFunctionType as Act

F32 = mybir.dt.float32
BF16 = mybir.dt.bfloat16
I16 = mybir.dt.int16

W = 128
C = 64
K2 = 9
PADW = 132


def _shape_consts(H):
    NPX = H * W
    NCH = 8
    CH = NPX // NCH          # pixels per chunk
    RPC = CH // W            # image rows per chunk
    S = CH // 16             # wrap cols per chunk slot
    PADH = H + 4
    NTOK = PADH * PADW
    NSPL = max(1, CH // 512)
    SPL = CH // NSPL         # matmul N per split (<=512)
    return NPX, NCH, CH, RPC, S, PADH, NTOK, NSPL, SPL


def build_nc(H=128, num_devices=8):
    NPX, NCH, CH, RPC, S, PADH, NTOK, NSPL, SPL = _shape_consts(H)
    RSPL = SPL // W                      # image rows per split
    NTOKP = ((NTOK + 2047) // 2048) * 2048

    es = ExitStack()
    nc = bacc.Bacc("TRN2", target_bir_lowering=False, debug=False,
                   num_devices=num_devices)

    x_img = nc.dram_tensor("x_img", [C, NPX], F32, kind="ExternalInput")
    x_cont = nc.dram_tensor("x_cont", [C, NPX], F32, kind="ExternalInput")
    out = nc.dram_tensor("out", [C, NPX], F32, kind="ExternalOutput")

    fuse_lhsT = nc.dram_tensor("fuse_lhsT", [128, 64], BF16, kind="ExternalInput")
    pw, sw = {}, {}
    for q in ("dy", "dx", "mod"):
        for ty in range(3):
            pw[(q, ty)] = nc.dram_tensor(f"pw_{q}_{ty}", [128, 32], BF16,
                                         kind="ExternalInput")
            sw[(q, ty)] = nc.dram_tensor(f"sw_{q}_{ty}", [64, 32], BF16,
                                         kind="ExternalInput")
    dcols = {}
    for nm in ("bias_dy", "bias_dx", "bias_mod", "xw_s1", "xw_s2"):
        dcols[nm] = nc.dram_tensor(nm, [128, 1], F32, kind="ExternalInput")
    bt0 = nc.dram_tensor("bt0", [128, 2 * CH], F32, kind="ExternalInput")
    btd = nc.dram_tensor("btd", [128, 2 * CH], F32, kind="ExternalInput")
    sel = nc.dram_tensor("sel", [128, K2 * 128], BF16, kind="ExternalInput")
    reg0 = nc.dram_tensor("reg0", [128, K2 * 64], BF16, kind="ExternalInput")
    reg1 = nc.dram_tensor("reg1", [128, K2 * 64], BF16, kind="ExternalInput")

    xtok = nc.dram_tensor("xtok", [NTOKP, 128], BF16, kind="Internal")
    xtok3 = xtok.ap()[0:NTOK, :].rearrange("(a b) e -> a b e", b=PADW)
    NPAIR_P = PADH // 2
    NPAIR_Q = PADH // 2 - 1
    QBASE = NPAIR_P * PADW
    NTOK2 = (NPAIR_P + NPAIR_Q) * PADW
    xtok2 = nc.dram_tensor("xtok2", [NTOK2, 256], BF16, kind="Internal")

    MM = lambda *a, **k: nc.tensor.matmul(*a, **k)

    with TileContext(nc) as tc:
        pconst = es.enter_context(tc.tile_pool(name="pconst", bufs=1))
        pp = es.enter_context(tc.tile_pool(name="pp", bufs=1))
        pv = es.enter_context(tc.tile_pool(name="pv", bufs=1))

        # ---- stage constants
        fuse_w = pconst.tile([128, 64], BF16)
        nc.sync.dma_start(fuse_w[:], fuse_lhsT.ap())
        conv_w = {}
        for q in ("dy", "dx", "mod"):
            for ty in range(3):
                tP = pconst.tile([128, 32], BF16, name=f"cwp_{q}{ty}")
                nc.sync.dma_start(tP[:], pw[(q, ty)].ap())
                tS = pconst.tile([64, 32], BF16, name=f"cws_{q}{ty}")
                nc.sync.dma_start(tS[:], sw[(q, ty)].ap())
                conv_w[(q, ty)] = (tP, tS)
        col = {}
        for nm in ("bias_dy", "bias_dx", "bias_mod", "xw_s1", "xw_s2"):
            t = pconst.tile([128, 1], F32, name=f"c_{nm}")
            nc.sync.dma_start(t[:], dcols[nm].ap())
            col[nm] = t
        btok0 = pconst.tile([128, 2 * CH], F32)
        nc.sync.dma_start(btok0[:], bt0.ap())
        btokd = pconst.tile([128, 2 * CH], F32)
        nc.sync.dma_start(btokd[:], btd.ap())
        sel_sb = pconst.tile([128, K2 * 128], BF16)
        nc.sync.dma_start(sel_sb[:], sel.ap())
        regsb = {}
        for y, t in ((0, reg0), (1, reg1)):
            r = pconst.tile([128, K2 * 64], BF16, name=f"regsb{y}")
            nc.sync.dma_start(r[:], t.ap())
            regsb[y] = r

        CY, IDXT = {}, {}
        WIDX = pp.tile([128, K2 * 8 * S], I16, name="widx")

        with tc.tile_pool(name="pX", bufs=1) as pX:
            X2 = pp.tile([128, PADH, PADW], BF16, name="X2")

            # =============== phase 0 ===============
            with tc.tile_pool(name="pin", bufs=1) as pin, \
                 tc.tile_pool(name="p0ps", bufs=2, space="PSUM") as p0ps:
                instk = pin.tile([128, NPX], BF16)
                nc.gpsimd.dma_start(instk[0:64, :], x_img.ap())
                nc.gpsimd.dma_start(instk[64:128, :], x_cont.ap())

                nc.vector.memset(X2[:, :, :], 0.0)
                zt = pin.tile([128, 2048], BF16)
                nc.vector.memset(zt[:, :], 0.0)
                # zero only the pad ring (interior is fully overwritten by
                # the token stores below; xtok2 derivation reads rows<NTOK)
                ring = [(0, 2 * PADW), (NTOK - 2 * PADW, 2 * PADW)]
                for r0, n in ring:
                    while n > 0:
                        c = min(128, n)
                        nc.sync.dma_start(xtok.ap()[r0:r0 + c, :], zt[0:c, 0:128])
                        r0 += c
                        n -= c
                # left/right 2-col strips, 64 rows per DMA
                for c0, wd in ((0, 2), (PADW - 3, 3)):
                    for rr in range(2, PADH - 2, 64):
                        nr = min(64, PADH - 2 - rr)
                        nc.sync.dma_start(xtok3[rr:rr + nr, c0:c0 + wd, :],
                                          zt[0:nr, 0:128 * wd])

                # fuse conv -> X2 rows 0-63 interior
                for c8 in range(NCH):
                    for j in range(NSPL):
                        ps = p0ps.tile([64, SPL], F32, tag="fuseps")
                        off = c8 * CH + j * SPL
                        MM(ps[:], fuse_w[:, :], instk[:, off:off + SPL],
                           start=True, stop=True)
                        i0 = off // W
                        nc.scalar.copy(X2[0:64, 2 + i0:2 + i0 + RSPL, 2:130],
                                       ps[:].rearrange("p (a b) -> p a b", b=W))

                # transposed fuse -> xtok tokens
                stg = pin.tile([128, RPC * 64], BF16, tag="stg", bufs=2)
                for c8 in range(NCH):
                    for r in range(RPC):
                        i = c8 * RPC + r
                        pst = p0ps.tile([128, 64], F32, tag="fuseT")
                        MM(pst[:], instk[:, i * W:(i + 1) * W], fuse_w[:, :],
                           start=True, stop=True)
                        nc.vector.tensor_copy(stg[:, r * 64:(r + 1) * 64], pst[:])
                    rr = c8 * RPC + 2
                    # first halves: token (y, x=2+j)[0:64] = pixel (y, 2+j)
                    nc.sync.dma_start(
                        xtok3[rr:rr + RPC, 2:130, 0:64].transpose([1, 0, 2]),
                        stg[:, :].rearrange("p (r e) -> p r e", e=64))
                    # second halves: token (y, x=1+j)[64:128] = pixel (y, 2+j)
                    nc.sync.dma_start(
                        xtok3[rr:rr + RPC, 1:129, 64:128].transpose([1, 0, 2]),
                        stg[:, :].rearrange("p (r e) -> p r e", e=64))

                # duplicate col-shifted copy on partitions 64-127 (per row-band
                # so convs can start before the whole fuse completes; the 2
                # skipped trailing elems per band are pad zeros on both sides)
                X2f = X2.rearrange("p a b -> p (a b)")
                band = [0] + [2 + c8 * RPC for c8 in range(1, NCH)] + [PADH]
                for bi in range(len(band) - 1):
                    r0, r1 = band[bi], band[bi + 1]
                    n = (r1 - r0) * PADW - 2
                    nc.sync.dma_start(X2f[64:128, r0 * PADW:r0 * PADW + n],
                                      X2f[0:64, r0 * PADW + 2:r0 * PADW + 2 + n])

            # derive 512B pair-row tokens: P copy (even y0), Q copy (odd y0)
            for par, npair, base in ((0, NPAIR_P, 0), (1, NPAIR_Q, QBASE)):
                for half in range(2):
                    nc.sync.dma_start(
                        xtok2.ap()[base:base + npair * PADW,
                                   128 * half:128 * half + 128]
                        .rearrange("(a b) e -> a b e", b=PADW),
                        xtok.ap()[(par + half) * PADW:
                                  (par + half) * PADW + npair * 2 * PADW, :]
                        .rearrange("(a c b) e -> a (c b) e", c=2, b=PADW)
                        [:, 0:PADW, :])

            # =============== phase A: convs + maps ===============
            with tc.tile_pool(name="paps", bufs=2, space="PSUM") as paps, \
                 tc.tile_pool(name="pam", bufs=1) as pam:
                for g in range(2):
                    qsb = {}
                    for q in ("dy", "dx", "mod"):
                        qps = paps.tile([128, CH], F32, tag="convps")
                        for cb in range(4):
                            c8 = g * 4 + cb
                            for j in range(NSPL):
                                ist = c8 * RPC + j * RSPL
                                dst = qps[32 * cb:32 * cb + 32,
                                          j * SPL:(j + 1) * SPL]
                                for ty in range(3):
                                    tP, tS = conv_w[(q, ty)]
                                    MM(dst,
                                       tP[:, :],
                                       X2[0:128, 1 + ist + ty:1 + ist + ty + RSPL,
                                          1:1 + W],
                                       start=(ty == 0), stop=False,
                                       tile_position=(0, 32 * cb))
                                    MM(dst,
                                       tS[:, :],
                                       X2[0:64, 1 + ist + ty:1 + ist + ty + RSPL,
                                          2:2 + W],
                                       start=False, stop=(ty == 2),
                                       tile_position=(0, 32 * cb))
                        qs = pam.tile([128, CH], BF16, tag=f"q_{q}",
                                      name=f"qsb_{q}{g}")
                        if q == "mod":
                            nc.scalar.activation(qs[:], qps[:], Act.Sigmoid,
                                                 bias=col["bias_mod"][:], scale=1.0)
                        else:
                            nc.scalar.activation(
                                qs[:], qps[:], Act.Identity,
                                bias=col["bias_dy" if q == "dy" else "bias_dx"][:],
                                scale=1.0)
                        qsb[q] = qs

                    FY = pam.tile([128, CH], BF16, tag="m1")
                    nc.vector.tensor_scalar(FY[:], qsb["dy"][:], 0.0, None, Op.is_lt)
                    FX = pam.tile([128, CH], BF16, tag="m2")
                    nc.vector.tensor_scalar(FX[:], qsb["dx"][:], 0.0, None, Op.is_lt)
                    RY = pam.tile([128, CH], BF16, tag="m3")
                    nc.vector.tensor_tensor(RY[:], qsb["dy"][:], FY[:], Op.add)
                    RX = pam.tile([128, CH], BF16, tag="m4")
                    nc.vector.tensor_tensor(RX[:], qsb["dx"][:], FX[:], Op.add)
                    XW = pam.tile([128, CH], BF16, tag="m5")
                    nc.vector.tensor_scalar(XW[:], RX[:], col["xw_s1"][:],
                                            col["xw_s2"][:], Op.mult, Op.add)
                    WY0N = pam.tile([128, CH], BF16, tag="m6")
                    nc.vector.scalar_tensor_tensor(WY0N[:], RY[:], 1.0,
                                                   qsb["mod"][:],
                                                   Op.subtract, Op.mult)
                    RYM = pam.tile([128, CH], BF16, tag="m7")
                    nc.vector.tensor_tensor(RYM[:], RY[:], qsb["mod"][:], Op.mult)
                    cy0 = pp.tile([128, CH], BF16, name=f"cy0_{g}")
                    nc.vector.tensor_tensor(cy0[:], WY0N[:], XW[:], Op.mult)
                    cy1 = pp.tile([128, CH], BF16, name=f"cy1_{g}")
                    nc.vector.tensor_tensor(cy1[:], RYM[:], XW[:], Op.mult)
                    CY[(g, 0)], CY[(g, 1)] = cy0, cy1

                    T1 = pam.tile([128, CH], F32, tag="m8")
                    nc.vector.tensor_tensor(T1[:], FY[:],
                                            btokd[:, g * CH:(g + 1) * CH], Op.mult)
                    TOK0 = pam.tile([128, CH], F32, tag="m9")
                    nc.vector.tensor_tensor(TOK0[:], btok0[:, g * CH:(g + 1) * CH],
                                            T1[:], Op.subtract)
                    T2 = pam.tile([128, CH], F32, tag="m8", name="T2")
                    nc.vector.tensor_tensor(T2[:], TOK0[:], FX[:], Op.subtract)
                    idx0 = pp.tile([128, CH], I16, name=f"idx0_{g}")
                    nc.vector.tensor_copy(idx0[:], T2[:])
                    IDXT[g] = idx0

                # wrapped indices: WIDX[y][p, slot*S + s] = IDX[row, p*S + s]
                for k in range(K2):
                    for g in range(2):
                        for cb in range(4):
                            slot = ((g * K2 + k) * 4 + cb) * S
                            sap = IDXT[g][32 * cb + k:32 * cb + k + 1, :]
                            eng = nc.sync if (k + cb) % 2 == 0 else nc.scalar
                            eng.dma_start(
                                WIDX[0:16, slot:slot + S],
                                sap.rearrange("p (a b) -> p a b", b=S))
                HW_ = K2 * 4 * S
                for g in range(2):
                    for r8 in range(1, 8):
                        nc.sync.dma_start(
                            WIDX[16 * r8:16 * r8 + 16, g * HW_:(g + 1) * HW_],
                            WIDX[0:16, g * HW_:(g + 1) * HW_])

        # =============== phase C: gather / weight / contract ===============
        import os as _os
        if _os.environ.get("SKIP_C"):
            es.close()
            nc.compile()
            return nc
        with tc.tile_pool(name="pcps", bufs=2, space="PSUM") as pcps, \
             tc.tile_pool(name="pops", bufs=1, space="PSUM") as pops, \
             tc.tile_pool(name="pc", bufs=3) as pc:
            for g in range(2):
                for hh in range(2):          # half-group: chunks (2hh, 2hh+1)
                    outp = pops.tile([128, CH], F32, tag="outp", bufs=1)
                    pend, CLAG = [], 6
                    for k in range(K2):
                        vb = {}
                        for ci2 in range(2):
                            vv = pv.tile([128, 2, CH], BF16, tag="vt", bufs=4,
                                         name=f"vv{ci2}")
                            islot = ((g * K2 + k) * 4 + 2 * hh + ci2) * S
                            if not _os.environ.get("NO_GATHER"):
                                nc.gpsimd.dma_gather(
                                    vv[:, :, :], xtok2.ap(),
                                    WIDX[:, islot:islot + S],
                                    num_idxs=CH, num_idxs_reg=CH,
                                    elem_size=256, transpose=True,
                                    single_packet=False)
                            vb[ci2] = vv
                        for y in (0, 1):
                            if _os.environ.get("NO_COMPUTE"):
                                continue
                            NH = NSPL                # 512-col groups
                            GW = CH // NH            # cols per group
                            NSUB = GW // SPL         # matmul splits per group
                            for ci in range(2):
                                cb = 2 * hh + ci
                                cy = CY[(g, y)]
                                cyv = cy[32 * cb:32 * cb + 18, :].rearrange(
                                    "p (a b) -> p b a", b=S)   # [18, S, 16]
                                for h in range(NH):
                                    wrepp = pcps.tile([128, GW], F32, tag="wrepp", bufs=4)
                                    for u in range(NSUB):
                                        q0 = (h * GW + u * SPL) // 16
                                        MM(wrepp[:, u * SPL:(u + 1) * SPL],
                                           sel_sb[32 * cb:32 * cb + 18,
                                                  k * 128:(k + 1) * 128],
                                           cyv[:, q0:q0 + SPL // 16, :],
                                           start=True, stop=True,
                                           tile_position=(32 * cb, 0),
                                           skip_group_check=True)
                                    wreps = pc.tile([128, GW], BF16, tag="wreps",
                                                    bufs=8)
                                    if (k + y + h) % 3 < 2:
                                        nc.scalar.copy(wreps[:], wrepp[:])
                                    else:
                                        nc.vector.tensor_copy(wreps[:], wrepp[:])
                                    prd = pc.tile([128, GW], BF16, tag="prd",
                                                  bufs=8)
                                    nc.vector.tensor_tensor(
                                        prd[:],
                                        vb[ci][:, y, h * GW:(h + 1) * GW],
                                        wreps[:], Op.mult)

                                    def _emit_contr(prd=prd, y=y, k=k, ci=ci,
                                                    h=h, outp=outp, GW=GW):
                                        for u in range(NSUB):
                                            MM(outp[64 * ci:64 * ci + 64,
                                                    h * GW + u * SPL:
                                                    h * GW + (u + 1) * SPL],
                                               regsb[y][:, k * 64:(k + 1) * 64],
                                               prd[:, u * SPL:(u + 1) * SPL],
                                               start=(k == 0 and y == 0),
                                               stop=(k == K2 - 1 and y == 1),
                                               skip_group_check=True)
                                    pend.append(_emit_contr)
                                    if len(pend) > CLAG:
                                        pend.pop(0)()
                    for fe in pend:
                        fe()
                    if _os.environ.get("NO_COMPUTE"):
                        nc.vector.memset(outp[:, :], 0.0)
                    for ci in range(2):
                        cb = 2 * hh + ci
                        c8 = g * 4 + cb
                        outs = pc.tile([64, CH], F32, tag="outs", bufs=2)
                        # out col m = p*S + q <- outp col n = q*16 + p
                        opv = outp[64 * ci:64 * ci + 64, :].rearrange(
                            "p (q a) -> p a q", a=16)       # [64, 16, S]
                        nc.scalar.copy(
                            outs[:].rearrange("p (a q) -> p a q", a=16), opv)
                        nc.sync.dma_start(out.ap()[:, c8 * CH:(c8 + 1) * CH],
                                          outs[:])
        es.close()

    nc.compile()
    return nc


# ======================= host-side preparation =======================

def _bf16(x):
    x = np.asarray(x, np.float32)
    u = x.view(np.uint32)
    r = ((u >> 16) + ((u >> 15) & 1)).astype(np.uint16)  # rne-ish
    return r


def _host_consts(w_fuse, w_off, b_off, w_mod, b_mod, w_reg, H=128):
    NPX, NCH, CH, RPC, S, PADH, NTOK, NSPL, SPL = _shape_consts(H)
    import ml_dtypes
    bf = lambda x: np.asarray(x, np.float32).astype(ml_dtypes.bfloat16)

    consts = {}
    wf = np.asarray(w_fuse, np.float32).reshape(64, 128)
    consts["fuse_lhsT"] = bf(np.ascontiguousarray(wf.T))

    w_off = np.asarray(w_off, np.float32).reshape(18, 64, 3, 3)
    w_mod = np.asarray(w_mod, np.float32).reshape(9, 64, 3, 3)

    def qw(q, k):
        return (w_off[2 * k] if q == "dy"
                else w_off[2 * k + 1] if q == "dx" else w_mod[k])

    for q in ("dy", "dx", "mod"):
        for ty in range(3):
            P = np.zeros((128, 32), np.float32)
            Sg = np.zeros((64, 32), np.float32)
            for m in range(18):
                k = m % 9
                P[0:64, m] = qw(q, k)[:, ty, 0]
                P[64:128, m] = qw(q, k)[:, ty, 2]
                Sg[0:64, m] = qw(q, k)[:, ty, 1]
            consts[f"pw_{q}_{ty}"] = bf(P)
            consts[f"sw_{q}_{ty}"] = bf(Sg)

    b_off = np.asarray(b_off, np.float32)
    b_mod = np.asarray(b_mod, np.float32)
    bdy = np.zeros((128, 1), np.float32)
    bdx = np.zeros((128, 1), np.float32)
    bmd = np.zeros((128, 1), np.float32)
    s1 = np.zeros((128, 1), np.float32)
    s2 = np.zeros((128, 1), np.float32)
    for r in range(128):
        rr = r % 32
        if rr < 18:
            k = rr % 9
            bdy[r] = b_off[2 * k]
            bdx[r] = b_off[2 * k + 1]
            bmd[r] = b_mod[k]
        if rr < 9:
            s1[r], s2[r] = -1.0, 1.0
        elif rr < 18:
            s1[r], s2[r] = 1.0, 0.0
    consts["bias_dy"], consts["bias_dx"], consts["bias_mod"] = bdy, bdx, bmd
    consts["xw_s1"], consts["xw_s2"] = s1, s2

    PADH_ = H + 4
    QBASE = (PADH_ // 2) * PADW

    def _pairtok(y0):
        return np.where(y0 % 2 == 0, (y0 // 2) * PADW,
                        QBASE + (y0 // 2) * PADW)

    b0 = np.zeros((128, 2 * CH), np.float32)
    bd = np.zeros((128, 2 * CH), np.float32)
    for r in range(128):
        cb = r // 32
        rr = r % 32
        k = rr % 9 if rr < 18 else 0
        ky, kx = k // 3, k % 3
        for g in range(2):
            c8 = g * 4 + cb
            cols = np.arange(CH)
            px = c8 * CH + cols
            i, j = px // W, px % W
            yb = i + 1 + ky
            pt0 = _pairtok(yb) + (j + 1 + kx)
            ptm = _pairtok(yb - 1) + (j + 1 + kx)
            b0[r, g * CH:(g + 1) * CH] = pt0
            bd[r, g * CH:(g + 1) * CH] = pt0 - ptm
    consts["bt0"] = b0
    consts["btd"] = bd

    selm = np.zeros((128, K2 * 128), np.float32)
    for cb in range(4):
        for k in range(K2):
            selm[32 * cb + k, k * 128:k * 128 + 64] = 1.0
            selm[32 * cb + k + 9, k * 128 + 64:k * 128 + 128] = 1.0
    consts["sel"] = bf(selm)

    w_reg = np.asarray(w_reg, np.float32).reshape(64, 64, 3, 3)
    r0 = np.zeros((128, K2 * 64), np.float32)
    r1 = np.zeros((128, K2 * 64), np.float32)
    for k in range(K2):
        ky, kx = k // 3, k % 3
        blkT = w_reg[:, :, ky, kx].T       # [c, o]
        r0[0:64, k * 64:(k + 1) * 64] = -2.0 * blkT
        r0[64:128, k * 64:(k + 1) * 64] = -2.0 * blkT
        r1[0:64, k * 64:(k + 1) * 64] = 2.0 * blkT
        r1[64:128, k * 64:(k + 1) * 64] = 2.0 * blkT
    consts["reg0"] = bf(r0)
    consts["reg1"] = bf(r1)
    return consts


_NC_CACHE = {}


def kernel(x_img, x_cont, w_fuse, w_off, b_off, w_mod, b_mod, w_reg):
    from concourse.bass_utils import run_bass_kernel_spmd

    H = 128
    B = int(x_img.shape[0])
    NPX = H * W
    if "nc" not in _NC_CACHE:
        _NC_CACHE["nc"] = build_nc(H=H, num_devices=8)
    nc = _NC_CACHE["nc"]

    consts = _host_consts(w_fuse, w_off, b_off, w_mod, b_mod, w_reg, H=H)
    x_img = np.asarray(x_img, np.float32)
    x_cont = np.asarray(x_cont, np.float32)
    in_maps = []
    for b in range(B):
        m = dict(consts)
        m["x_img"] = np.ascontiguousarray(x_img[b].reshape(C, NPX))
        m["x_cont"] = np.ascontiguousarray(x_cont[b].reshape(C, NPX))
        in_maps.append(m)

    res = run_bass_kernel_spmd(nc, in_maps, core_ids=list(range(B)))
    outs = [np.asarray(res.results[b]["out"], np.float32).reshape(C, H, W)
            for b in range(B)]
    return np.stack(outs)


# revision 30
# speedup vs baseline: 1.3551x; 1.0018x over previous
"""Modulated deformable conv (DCNv2-style) Trainium2 Bass kernel.

Batch data-parallel over 8 NeuronCores (1 batch element per core).

Per-core pipeline:
  1. fuse 1x1 conv (PE)  -> x, kept as CHW padded in SBUF (X2, with a
     col-shifted duplicate on partitions 64-127 for K-stacked conv taps)
     and as NHWC 2-pixel tokens in HBM (xtok) for gathering.
  2. dy/dx/mod 3x3 convs (PE, 3 pair-slots K=128 + 3 single-slots K=64).
  3. map pipeline (DVE/ACT): floor flags, fracs, modulated corner-weight
     maps CY0/CY1, int16 gather token indices.
  4. dma_gather (transpose=True, 256B tokens = 2px x 64ch bf16): for each
     kernel tap k and corner row y, V[128=(2px,64ch), npix].
  5. corner weights replicated across 128 partitions via PE selector
     matmuls; prod = V * Wrep (DVE); final contraction on PE with w_reg
     folded lhsT (sign/2x-sigmoid folded in host-side).

Column ordering note: gather list position n maps to map-column
sigma(n) = (n%16)*S + n//16  (S = chunk_cols/16) because indices are
stored 16-partition-wrapped with a contiguous inner dim. sigma is applied
at 3 AP sites: the index wrap DMA, the repl-matmul rhs view, and the
final PSUM->SBUF unpermute copy.
"""
import sys

sys.path.insert(0, "/opt/trn_rl_repo")

from contextlib import ExitStack

import numpy as np

import concourse.bass as bass
import concourse.bacc as bacc
import concourse.mybir as mybir
from concourse.tile import TileContext
from concourse.mybir import AluOpType as Op
from concourse.mybir import ActivationFunctionType as Act

F32 = mybir.dt.float32
BF16 = mybir.dt.bfloat16
I16 = mybir.dt.int16

W = 128
C = 64
K2 = 9
PADW = 132


def _shape_consts(H):
    NPX = H * W
    NCH = 8
    CH = NPX // NCH          # pixels per chunk
    RPC = CH // W            # image rows per chunk
    S = CH // 16             # wrap cols per chunk slot
    PADH = H + 4
    NTOK = PADH * PADW
    NSPL = max(1, CH // 512)
    SPL = CH // NSPL         # matmul N per split (<=512)
    return NPX, NCH, CH, RPC, S, PADH, NTOK, NSPL, SPL


def build_nc(H=128, num_devices=8):
    NPX, NCH, CH, RPC, S, PADH, NTOK, NSPL, SPL = _shape_consts(H)
    RSPL = SPL // W                      # image rows per split
    NTOKP = ((NTOK + 2047) // 2048) * 2048

    es = ExitStack()
    nc = bacc.Bacc("TRN2", target_bir_lowering=False, debug=False,
                   num_devices=num_devices)

    x_img = nc.dram_tensor("x_img", [C, NPX], F32, kind="ExternalInput")
    x_cont = nc.dram_tensor("x_cont", [C, NPX], F32, kind="ExternalInput")
    out = nc.dram_tensor("out", [C, NPX], F32, kind="ExternalOutput")

    fuse_lhsT = nc.dram_tensor("fuse_lhsT", [128, 64], BF16, kind="ExternalInput")
    pw, sw = {}, {}
    for q in ("dy", "dx", "mod"):
        for ty in range(3):
            pw[(q, ty)] = nc.dram_tensor(f"pw_{q}_{ty}", [128, 32], BF16,
                                         kind="ExternalInput")
            sw[(q, ty)] = nc.dram_tensor(f"sw_{q}_{ty}", [64, 32], BF16,
                                         kind="ExternalInput")
    dcols = {}
    for nm in ("bias_dy", "bias_dx", "bias_mod", "xw_s1", "xw_s2"):
        dcols[nm] = nc.dram_tensor(nm, [128, 1], F32, kind="ExternalInput")
    bt0 = nc.dram_tensor("bt0", [128, 2 * CH], F32, kind="ExternalInput")
    btd = nc.dram_tensor("btd", [128, 2 * CH], F32, kind="ExternalInput")
    sel = nc.dram_tensor("sel", [128, K2 * 128], BF16, kind="ExternalInput")
    reg0 = nc.dram_tensor("reg0", [128, K2 * 64], BF16, kind="ExternalInput")
    reg1 = nc.dram_tensor("reg1", [128, K2 * 64], BF16, kind="ExternalInput")

    xtok = nc.dram_tensor("xtok", [NTOKP, 128], BF16, kind="Internal")
    xtok3 = xtok.ap()[0:NTOK, :].rearrange("(a b) e -> a b e", b=PADW)
    NPAIR_P = PADH // 2
    NPAIR_Q = PADH // 2 - 1
    QBASE = NPAIR_P * PADW
    NTOK2 = (NPAIR_P + NPAIR_Q) * PADW
    xtok2 = nc.dram_tensor("xtok2", [NTOK2, 256], BF16, kind="Internal")

    MM = lambda *a, **k: nc.tensor.matmul(*a, **k)

    with TileContext(nc) as tc:
        pconst = es.enter_context(tc.tile_pool(name="pconst", bufs=1))
        pp = es.enter_context(tc.tile_pool(name="pp", bufs=1))
        pv = es.enter_context(tc.tile_pool(name="pv", bufs=1))

        # ---- stage constants
        fuse_w = pconst.tile([128, 64], BF16)
        nc.sync.dma_start(fuse_w[:], fuse_lhsT.ap())
        conv_w = {}
        for q in ("dy", "dx", "mod"):
            for ty in range(3):
                tP = pconst.tile([128, 32], BF16, name=f"cwp_{q}{ty}")
                nc.sync.dma_start(tP[:], pw[(q, ty)].ap())
                tS = pconst.tile([64, 32], BF16, name=f"cws_{q}{ty}")
                nc.sync.dma_start(tS[:], sw[(q, ty)].ap())
                conv_w[(q, ty)] = (tP, tS)
        col = {}
        for nm in ("bias_dy", "bias_dx", "bias_mod", "xw_s1", "xw_s2"):
            t = pconst.tile([128, 1], F32, name=f"c_{nm}")
            nc.sync.dma_start(t[:], dcols[nm].ap())
            col[nm] = t
        btok0 = pconst.tile([128, 2 * CH], F32)
        nc.sync.dma_start(btok0[:], bt0.ap())
        btokd = pconst.tile([128, 2 * CH], F32)
        nc.sync.dma_start(btokd[:], btd.ap())
        sel_sb = pconst.tile([128, K2 * 128], BF16)
        nc.sync.dma_start(sel_sb[:], sel.ap())
        regsb = {}
        for y, t in ((0, reg0), (1, reg1)):
            r = pconst.tile([128, K2 * 64], BF16, name=f"regsb{y}")
            nc.sync.dma_start(r[:], t.ap())
            regsb[y] = r

        CY, IDXT = {}, {}
        WIDX = pp.tile([128, K2 * 8 * S], I16, name="widx")

        with tc.tile_pool(name="pX", bufs=1) as pX:
            X2 = pp.tile([128, PADH, PADW], BF16, name="X2")

            # =============== phase 0 ===============
            with tc.tile_pool(name="pin", bufs=1) as pin, \
                 tc.tile_pool(name="p0ps", bufs=2, space="PSUM") as p0ps:
                instk = pin.tile([128, NPX], BF16)
                nc.gpsimd.dma_start(instk[0:64, :], x_img.ap())
                nc.gpsimd.dma_start(instk[64:128, :], x_cont.ap())

                nc.vector.memset(X2[:, :, :], 0.0)
                zt = pin.tile([128, 2048], BF16)
                nc.vector.memset(zt[:, :], 0.0)
                # zero only the pad ring (interior is fully overwritten by
                # the token stores below; xtok2 derivation reads rows<NTOK)
                ring = [(0, 2 * PADW), (NTOK - 2 * PADW, 2 * PADW)]
                for r0, n in ring:
                    while n > 0:
                        c = min(128, n)
                        nc.sync.dma_start(xtok.ap()[r0:r0 + c, :], zt[0:c, 0:128])
                        r0 += c
                        n -= c
                # left/right 2-col strips, 64 rows per DMA
                for c0, wd in ((0, 2), (PADW - 3, 3)):
                    for rr in range(2, PADH - 2, 64):
                        nr = min(64, PADH - 2 - rr)
                        nc.sync.dma_start(xtok3[rr:rr + nr, c0:c0 + wd, :],
                                          zt[0:nr, 0:128 * wd])

                # fuse conv -> X2 rows 0-63 interior
                for c8 in range(NCH):
                    for j in range(NSPL):
                        ps = p0ps.tile([64, SPL], F32, tag="fuseps")
                        off = c8 * CH + j * SPL
                        MM(ps[:], fuse_w[:, :], instk[:, off:off + SPL],
                           start=True, stop=True)
                        i0 = off // W
                        ps3 = ps[:].rearrange("p (a b) -> p a b", b=W)
                        nc.scalar.copy(X2[0:64, 2 + i0:2 + i0 + RSPL, 2:130], ps3)
                        # dup rows hold x_pad shifted +2 cols: same psum data
                        # lands at col 0 (X2[64+c,i,b] = x_pad[c,i,b+2])
                        nc.scalar.copy(X2[64:128, 2 + i0:2 + i0 + RSPL, 0:128], ps3)

                # transposed fuse -> xtok tokens
                stg = pin.tile([128, RPC * 64], BF16, tag="stg", bufs=2)
                for c8 in range(NCH):
                    for r in range(RPC):
                        i = c8 * RPC + r
                        pst = p0ps.tile([128, 64], F32, tag="fuseT")
                        MM(pst[:], instk[:, i * W:(i + 1) * W], fuse_w[:, :],
                           start=True, stop=True)
                        nc.vector.tensor_copy(stg[:, r * 64:(r + 1) * 64], pst[:])
                    rr = c8 * RPC + 2
                    # first halves: token (y, x=2+j)[0:64] = pixel (y, 2+j)
                    nc.sync.dma_start(
                        xtok3[rr:rr + RPC, 2:130, 0:64].transpose([1, 0, 2]),
                        stg[:, :].rearrange("p (r e) -> p r e", e=64))
                    # second halves: token (y, x=1+j)[64:128] = pixel (y, 2+j)
                    nc.sync.dma_start(
                        xtok3[rr:rr + RPC, 1:129, 64:128].transpose([1, 0, 2]),
                        stg[:, :].rearrange("p (r e) -> p r e", e=64))

    
            # derive 512B pair-row tokens: P copy (even y0), Q copy (odd y0)
            for par, npair, base in ((0, NPAIR_P, 0), (1, NPAIR_Q, QBASE)):
                for half in range(2):
                    nc.sync.dma_start(
                        xtok2.ap()[base:base + npair * PADW,
                                   128 * half:128 * half + 128]
                        .rearrange("(a b) e -> a b e", b=PADW),
                        xtok.ap()[(par + half) * PADW:
                                  (par + half) * PADW + npair * 2 * PADW, :]
                        .rearrange("(a c b) e -> a (c b) e", c=2, b=PADW)
                        [:, 0:PADW, :])

            # =============== phase A: convs + maps ===============
            with tc.tile_pool(name="paps", bufs=2, space="PSUM") as paps, \
                 tc.tile_pool(name="pam", bufs=1) as pam:
                for g in range(2):
                    qsb = {}
                    for q in ("dy", "dx", "mod"):
                        qps = paps.tile([128, CH], F32, tag="convps")
                        for cb in range(4):
                            c8 = g * 4 + cb
                            for j in range(NSPL):
                                ist = c8 * RPC + j * RSPL
                                dst = qps[32 * cb:32 * cb + 32,
                                          j * SPL:(j + 1) * SPL]
                                for ty in range(3):
                                    tP, tS = conv_w[(q, ty)]
                                    MM(dst,
                                       tP[:, :],
                                       X2[0:128, 1 + ist + ty:1 + ist + ty + RSPL,
                                          1:1 + W],
                                       start=(ty == 0), stop=False,
                                       tile_position=(0, 32 * cb))
                                    MM(dst,
                                       tS[:, :],
                                       X2[0:64, 1 + ist + ty:1 + ist + ty + RSPL,
                                          2:2 + W],
                                       start=False, stop=(ty == 2),
                                       tile_position=(0, 32 * cb))
                        qs = pam.tile([128, CH], BF16, tag=f"q_{q}",
                                      name=f"qsb_{q}{g}")
                        if q == "mod":
                            nc.scalar.activation(qs[:], qps[:], Act.Sigmoid,
                                                 bias=col["bias_mod"][:], scale=1.0)
                        else:
                            nc.scalar.activation(
                                qs[:], qps[:], Act.Identity,
                                bias=col["bias_dy" if q == "dy" else "bias_dx"][:],
                                scale=1.0)
                        qsb[q] = qs

                    FY = pam.tile([128, CH], BF16, tag="m1")
                    nc.vector.tensor_scalar(FY[:], qsb["dy"][:], 0.0, None, Op.is_lt)
                    FX = pam.tile([128, CH], BF16, tag="m2")
                    nc.vector.tensor_scalar(FX[:], qsb["dx"][:], 0.0, None, Op.is_lt)
                    RY = pam.tile([128, CH], BF16, tag="m3")
                    nc.vector.tensor_tensor(RY[:], qsb["dy"][:], FY[:], Op.add)
                    RX = pam.tile([128, CH], BF16, tag="m4")
                    nc.vector.tensor_tensor(RX[:], qsb["dx"][:], FX[:], Op.add)
                    XW = pam.tile([128, CH], BF16, tag="m5")
                    nc.vector.tensor_scalar(XW[:], RX[:], col["xw_s1"][:],
                                            col["xw_s2"][:], Op.mult, Op.add)
                    WY0N = pam.tile([128, CH], BF16, tag="m6")
                    nc.vector.scalar_tensor_tensor(WY0N[:], RY[:], 1.0,
                                                   qsb["mod"][:],
                                                   Op.subtract, Op.mult)
                    RYM = pam.tile([128, CH], BF16, tag="m7")
                    nc.vector.tensor_tensor(RYM[:], RY[:], qsb["mod"][:], Op.mult)
                    cy0 = pp.tile([128, CH], BF16, name=f"cy0_{g}")
                    nc.vector.tensor_tensor(cy0[:], WY0N[:], XW[:], Op.mult)
                    cy1 = pp.tile([128, CH], BF16, name=f"cy1_{g}")
                    nc.vector.tensor_tensor(cy1[:], RYM[:], XW[:], Op.mult)
                    CY[(g, 0)], CY[(g, 1)] = cy0, cy1

                    T1 = pam.tile([128, CH], F32, tag="m8")
                    nc.vector.tensor_tensor(T1[:], FY[:],
                                            btokd[:, g * CH:(g + 1) * CH], Op.mult)
                    TOK0 = pam.tile([128, CH], F32, tag="m9")
                    nc.vector.tensor_tensor(TOK0[:], btok0[:, g * CH:(g + 1) * CH],
                                            T1[:], Op.subtract)
                    T2 = pam.tile([128, CH], F32, tag="m8", name="T2")
                    nc.vector.tensor_tensor(T2[:], TOK0[:], FX[:], Op.subtract)
                    idx0 = pp.tile([128, CH], I16, name=f"idx0_{g}")
                    nc.vector.tensor_copy(idx0[:], T2[:])
                    IDXT[g] = idx0

                # wrapped indices: WIDX[y][p, slot*S + s] = IDX[row, p*S + s]
                for k in range(K2):
                    for g in range(2):
                        for cb in range(4):
                            slot = ((g * K2 + k) * 4 + cb) * S
                            sap = IDXT[g][32 * cb + k:32 * cb + k + 1, :]
                            eng = nc.sync if (k + cb) % 2 == 0 else nc.scalar
                            eng.dma_start(
                                WIDX[0:16, slot:slot + S],
                                sap.rearrange("p (a b) -> p a b", b=S))
                HW_ = K2 * 4 * S
                for g in range(2):
                    for r8 in range(1, 8):
                        nc.sync.dma_start(
                            WIDX[16 * r8:16 * r8 + 16, g * HW_:(g + 1) * HW_],
                            WIDX[0:16, g * HW_:(g + 1) * HW_])

        # =============== phase C: gather / weight / contract ===============
        import os as _os
        if _os.environ.get("SKIP_C"):
            es.close()
            nc.compile()
            return nc
        with tc.tile_pool(name="pcps", bufs=2, space="PSUM") as pcps, \
             tc.tile_pool(name="pops", bufs=1, space="PSUM") as pops, \
             tc.tile_pool(name="pc", bufs=3) as pc:
            for g in range(2):
                for hh in range(2):          # half-group: chunks (2hh, 2hh+1)
                    outp = pops.tile([128, CH], F32, tag="outp", bufs=1)
                    pend, CLAG = [], 6
                    for k in range(K2):
                        vb = {}
                        for ci2 in range(2):
                            vv = pv.tile([128, 2, CH], BF16, tag="vt", bufs=4,
                                         name=f"vv{ci2}")
                            islot = ((g * K2 + k) * 4 + 2 * hh + ci2) * S
                            if not _os.environ.get("NO_GATHER"):
                                nc.gpsimd.dma_gather(
                                    vv[:, :, :], xtok2.ap(),
                                    WIDX[:, islot:islot + S],
                                    num_idxs=CH, num_idxs_reg=CH,
                                    elem_size=256, transpose=True,
                                    single_packet=False)
                            vb[ci2] = vv
                        for y in (0, 1):
                            if _os.environ.get("NO_COMPUTE"):
                                continue
                            NH = NSPL                # 512-col groups
                            GW = CH // NH            # cols per group
                            NSUB = GW // SPL         # matmul splits per group
                            for ci in range(2):
                                cb = 2 * hh + ci
                                cy = CY[(g, y)]
                                cyv = cy[32 * cb:32 * cb + 18, :].rearrange(
                                    "p (a b) -> p b a", b=S)   # [18, S, 16]
                                for h in range(NH):
                                    wrepp = pcps.tile([128, GW], F32, tag="wrepp", bufs=4)
                                    for u in range(NSUB):
                                        q0 = (h * GW + u * SPL) // 16
                                        MM(wrepp[:, u * SPL:(u + 1) * SPL],
                                           sel_sb[32 * cb:32 * cb + 18,
                                                  k * 128:(k + 1) * 128],
                                           cyv[:, q0:q0 + SPL // 16, :],
                                           start=True, stop=True,
                                           tile_position=(32 * cb, 0),
                                           skip_group_check=True)
                                    wreps = pc.tile([128, GW], BF16, tag="wreps",
                                                    bufs=8)
                                    if (k + y + h) % 3 < 2:
                                        nc.scalar.copy(wreps[:], wrepp[:])
                                    else:
                                        nc.vector.tensor_copy(wreps[:], wrepp[:])
                                    prd = pc.tile([128, GW], BF16, tag="prd",
                                                  bufs=8)
                                    nc.vector.tensor_tensor(
                                        prd[:],
                                        vb[ci][:, y, h * GW:(h + 1) * GW],
                                        wreps[:], Op.mult)

                                    def _emit_contr(prd=prd, y=y, k=k, ci=ci,
                                                    h=h, outp=outp, GW=GW):
                                        for u in range(NSUB):
                                            MM(outp[64 * ci:64 * ci + 64,
                                                    h * GW + u * SPL:
                                                    h * GW + (u + 1) * SPL],
                                               regsb[y][:, k * 64:(k + 1) * 64],
                                               prd[:, u * SPL:(u + 1) * SPL],
                                               start=(k == 0 and y == 0),
                                               stop=(k == K2 - 1 and y == 1),
                                               skip_group_check=True)
                                    pend.append(_emit_contr)
                                    if len(pend) > CLAG:
                                        pend.pop(0)()
                    for fe in pend:
                        fe()
                    if _os.environ.get("NO_COMPUTE"):
                        nc.vector.memset(outp[:, :], 0.0)
                    for ci in range(2):
                        cb = 2 * hh + ci
                        c8 = g * 4 + cb
                        outs = pc.tile([64, CH], F32, tag="outs", bufs=2)
                        # out col m = p*S + q <- outp col n = q*16 + p
                        opv = outp[64 * ci:64 * ci + 64, :].rearrange(
                            "p (q a) -> p a q", a=16)       # [64, 16, S]
                        nc.scalar.copy(
                            outs[:].rearrange("p (a q) -> p a q", a=16), opv)
                        nc.sync.dma_start(out.ap()[:, c8 * CH:(c8 + 1) * CH],
                                          outs[:])
        es.close()

    nc.compile()
    return nc


# ======================= host-side preparation =======================

def _bf16(x):
    x = np.asarray(x, np.float32)
    u = x.view(np.uint32)
    r = ((u >> 16) + ((u >> 15) & 1)).astype(np.uint16)  # rne-ish
    return r


def _host_consts(w_fuse, w_off, b_off, w_mod, b_mod, w_reg, H=128):
    NPX, NCH, CH, RPC, S, PADH, NTOK, NSPL, SPL = _shape_consts(H)
    import ml_dtypes
    bf = lambda x: np.asarray(x, np.float32).astype(ml_dtypes.bfloat16)

    consts = {}
    wf = np.asarray(w_fuse, np.float32).reshape(64, 128)
    consts["fuse_lhsT"] = bf(np.ascontiguousarray(wf.T))

    w_off = np.asarray(w_off, np.float32).reshape(18, 64, 3, 3)
    w_mod = np.asarray(w_mod, np.float32).reshape(9, 64, 3, 3)

    def qw(q, k):
        return (w_off[2 * k] if q == "dy"
                else w_off[2 * k + 1] if q == "dx" else w_mod[k])

    for q in ("dy", "dx", "mod"):
        for ty in range(3):
            P = np.zeros((128, 32), np.float32)
            Sg = np.zeros((64, 32), np.float32)
            for m in range(18):
                k = m % 9
                P[0:64, m] = qw(q, k)[:, ty, 0]
                P[64:128, m] = qw(q, k)[:, ty, 2]
                Sg[0:64, m] = qw(q, k)[:, ty, 1]
            consts[f"pw_{q}_{ty}"] = bf(P)
            consts[f"sw_{q}_{ty}"] = bf(Sg)

    b_off = np.asarray(b_off, np.float32)
    b_mod = np.asarray(b_mod, np.float32)
    bdy = np.zeros((128, 1), np.float32)
    bdx = np.zeros((128, 1), np.float32)
    bmd = np.zeros((128, 1), np.float32)
    s1 = np.zeros((128, 1), np.float32)
    s2 = np.zeros((128, 1), np.float32)
    for r in range(128):
        rr = r % 32
        if rr < 18:
            k = rr % 9
            bdy[r] = b_off[2 * k]
            bdx[r] = b_off[2 * k + 1]
            bmd[r] = b_mod[k]
        if rr < 9:
            s1[r], s2[r] = -1.0, 1.0
        elif rr < 18:
            s1[r], s2[r] = 1.0, 0.0
    consts["bias_dy"], consts["bias_dx"], consts["bias_mod"] = bdy, bdx, bmd
    consts["xw_s1"], consts["xw_s2"] = s1, s2

    PADH_ = H + 4
    QBASE = (PADH_ // 2) * PADW

    def _pairtok(y0):
        return np.where(y0 % 2 == 0, (y0 // 2) * PADW,
                        QBASE + (y0 // 2) * PADW)

    b0 = np.zeros((128, 2 * CH), np.float32)
    bd = np.zeros((128, 2 * CH), np.float32)
    for r in range(128):
        cb = r // 32
        rr = r % 32
        k = rr % 9 if rr < 18 else 0
        ky, kx = k // 3, k % 3
        for g in range(2):
            c8 = g * 4 + cb
            cols = np.arange(CH)
            px = c8 * CH + cols
            i, j = px // W, px % W
            yb = i + 1 + ky
            pt0 = _pairtok(yb) + (j + 1 + kx)
            ptm = _pairtok(yb - 1) + (j + 1 + kx)
            b0[r, g * CH:(g + 1) * CH] = pt0
            bd[r, g * CH:(g + 1) * CH] = pt0 - ptm
    consts["bt0"] = b0
    consts["btd"] = bd

    selm = np.zeros((128, K2 * 128), np.float32)
    for cb in range(4):
        for k in range(K2):
            selm[32 * cb + k, k * 128:k * 128 + 64] = 1.0
            selm[32 * cb + k + 9, k * 128 + 64:k * 128 + 128] = 1.0
    consts["sel"] = bf(selm)

    w_reg = np.asarray(w_reg, np.float32).reshape(64, 64, 3, 3)
    r0 = np.zeros((128, K2 * 64), np.float32)
    r1 = np.zeros((128, K2 * 64), np.float32)
    for k in range(K2):
        ky, kx = k // 3, k % 3
        blkT = w_reg[:, :, ky, kx].T       # [c, o]
        r0[0:64, k * 64:(k + 1) * 64] = -2.0 * blkT
        r0[64:128, k * 64:(k + 1) * 64] = -2.0 * blkT
        r1[0:64, k * 64:(k + 1) * 64] = 2.0 * blkT
        r1[64:128, k * 64:(k + 1) * 64] = 2.0 * blkT
    consts["reg0"] = bf(r0)
    consts["reg1"] = bf(r1)
    return consts


_NC_CACHE = {}


def kernel(x_img, x_cont, w_fuse, w_off, b_off, w_mod, b_mod, w_reg):
    from concourse.bass_utils import run_bass_kernel_spmd

    H = 128
    B = int(x_img.shape[0])
    NPX = H * W
    if "nc" not in _NC_CACHE:
        _NC_CACHE["nc"] = build_nc(H=H, num_devices=8)
    nc = _NC_CACHE["nc"]

    consts = _host_consts(w_fuse, w_off, b_off, w_mod, b_mod, w_reg, H=H)
    x_img = np.asarray(x_img, np.float32)
    x_cont = np.asarray(x_cont, np.float32)
    in_maps = []
    for b in range(B):
        m = dict(consts)
        m["x_img"] = np.ascontiguousarray(x_img[b].reshape(C, NPX))
        m["x_cont"] = np.ascontiguousarray(x_cont[b].reshape(C, NPX))
        in_maps.append(m)

    res = run_bass_kernel_spmd(nc, in_maps, core_ids=list(range(B)))
    outs = [np.asarray(res.results[b]["out"], np.float32).reshape(C, H, W)
            for b in range(B)]
    return np.stack(outs)
